# revision 47
# baseline (speedup 1.0000x reference)
"""Trainium2 Bass kernel for nn_MoEPolicy (moe_routing).

Strategy (8 NeuronCores, SPMD, no collectives):
  - 32 graphs -> 4 graphs per core; each graph padded to 768 node slots
    (3072 padded node slots per core, 24 windows of 128).
  - Kernel 1 (per core): edge aggregation via one-hot PSUM matmuls (bf16
    one-hot scaled by edge_attr), v_emb (relu+LN), struct-token attention
    (batched, no per-node softmax max-subtract: scores are < 0.02 in
    magnitude), masked pooling, gating logits.  All heavy elementwise work
    batched into [128, 512] group ops; single activation table set
    (Ln/Exp/Relu/Square) -> one table load.
  - Host: top-4 expert selection per graph from device logits (index
    selection only), slices expert weights per core.
  - Kernel 2 (per core): route weights on device, two-pass expert
    pipeline: pass A computes all 24 expert chunk outputs (gelu on the
    scalar engine, bf16 matmuls), variances batched into one [24, 768]
    PSUM tile via selector-matmuls; one Ln+Exp gives all rstd rows; pass B
    broadcasts rstd*(route weight) via masked rank-24 matmuls and
    accumulates into the residual; task head.
All floating-point model math runs on device; the host only shards, pads,
permutes, selects indices, and casts dtypes.
"""

import sys

for _p in ("/opt/trn_rl_repo",):
    if _p not in sys.path:
        sys.path.insert(0, _p)

import numpy as np
import ml_dtypes

import concourse.bacc as bacc
import concourse.mybir as mybir
import concourse.tile as tile
from concourse.bass_utils import run_bass_kernel_spmd

F32 = mybir.dt.float32
F32R = mybir.dt.float32r
BF16 = mybir.dt.bfloat16
AF = mybir.ActivationFunctionType
ALU = mybir.AluOpType
AX = mybir.AxisListType
BF = ml_dtypes.bfloat16

# problem constants
D = 128
TD = 128
T = 64
NE = 16
KS = 2
TOPK = 4
TEMP = 0.6
B = 32
M = 10000
N = 20000
E = 160000
CF, VF, EF = 4, 6, 1

NCORE = 8
GPC = B // NCORE            # graphs per core
PAD_G = 768                 # node slots per graph
NC_NODES = GPC * PAD_G      # 3072
WPG = PAD_G // 128          # windows per graph (6)
NWIN = GPC * WPG            # 24 windows per core
NGRP = NWIN // 4            # 6 groups of 4 windows
LN_EPS = 1e-5
ISQ_TD = 1.0 / float(np.sqrt(np.float32(TD)))
CF1 = CF + 1

NSLOT = GPC * TOPK          # 16 dedicated (graph, k) slots per core
NCH = NSLOT + KS * GPC      # 24 chunk-slots (16 ded + 2 shared x 4 graphs)
HF = PAD_G // 2             # 384

CORE_IDS = list(range(NCORE))


# ---------------------------------------------------------------- host plan

def _plan(edge_cons, edge_vars, edge_attr, batch_idx):
    """Node slot assignment + edge window schedule. Pure index work."""
    order = np.argsort(batch_idx, kind="stable")
    bs = batch_idx[order]
    deg = np.bincount(edge_vars, minlength=N)

    node_of_slot = -np.ones((NCORE, NC_NODES), dtype=np.int64)
    slot_of_node = np.empty(N, dtype=np.int64)       # global slot = core*NC + s
    counts = np.zeros((NCORE, GPC), dtype=np.int64)  # real nodes per graph

    for g in range(B):
        nodes = order[np.searchsorted(bs, g, side="left"):
                      np.searchsorted(bs, g, side="right")]
        core, lg = g // GPC, g % GPC
        counts[core, lg] = len(nodes)
        if len(nodes) > PAD_G:
            raise RuntimeError(f"graph {g} has {len(nodes)} nodes > PAD_G={PAD_G}")
        # balance edge load across the graph's WPG windows
        nds = nodes[np.argsort(-deg[nodes], kind="stable")]
        wload = np.zeros(WPG, dtype=np.int64)
        wfill = np.zeros(WPG, dtype=np.int64)
        base = lg * PAD_G
        for nd in nds:
            cand = np.where(wfill < 128)[0]
            w = cand[np.argmin(wload[cand])]
            s = base + w * 128 + wfill[w]
            node_of_slot[core, s] = nd
            slot_of_node[nd] = core * NC_NODES + s
            wload[w] += deg[nd]
            wfill[w] += 1

    # edges -> (core, window, lane j)
    eslot = slot_of_node[edge_vars]
    ecore = eslot // NC_NODES
    es = eslot % NC_NODES
    ewin = es // 128
    ej = es % 128

    # tiles per window position, shared across cores
    cw = np.zeros((NCORE, NWIN), dtype=np.int64)
    per = {}
    for c in range(NCORE):
        sel = np.where(ecore == c)[0]
        for w in range(NWIN):
            ews = sel[ewin[sel] == w]
            per[(c, w)] = ews
            cw[c, w] = max(1, -(-len(ews) // 128))
    CW = cw.max(axis=0)
    ntot = int(CW.sum())

    ecidx = np.zeros((NCORE, 128 * ntot), dtype=np.int64)   # cons index per slot
    used = np.zeros((NCORE, 128 * ntot), dtype=bool)
    vloc = np.full((NCORE, 128 * ntot), -1.0, dtype=np.float32)
    eav = np.zeros((NCORE, 128 * ntot), dtype=np.float32)
    offs = np.concatenate([[0], np.cumsum(CW)]) * 128
    ea_flat = edge_attr.reshape(-1).astype(np.float32)
    for c in range(NCORE):
        for w in range(NWIN):
            ews = per[(c, w)]
            o = offs[w]
            ecidx[c, o:o + len(ews)] = edge_cons[ews]
            used[c, o:o + len(ews)] = True
            vloc[c, o:o + len(ews)] = ej[ews]
            eav[c, o:o + len(ews)] = ea_flat[ews]

    return dict(node_of_slot=node_of_slot, counts=counts, CW=CW.tolist(),
                ntot=ntot, ecidx=ecidx, used=used, vloc=vloc, eav=eav)


def _build_oea(plan, c):
    """One-hot (scaled by edge_attr) [128 lanes, tile, 128 nodes], bf16."""
    ntot = plan["ntot"]
    vloc = plan["vloc"][c].reshape(ntot, 128)
    eav = plan["eav"][c].reshape(ntot, 128)
    arr = np.zeros((128, ntot, 128), np.float32)   # [lane, tile, n]
    t_i, p_i = np.nonzero(vloc >= 0)
    arr[p_i, t_i, vloc[t_i, p_i].astype(np.int64)] = eav[t_i, p_i]
    return np.ascontiguousarray(arr.reshape(128, ntot * 128)).astype(BF)


def _sel24():
    """[128, 24, 24] bf16: SEL24[:, w, j] = (j == w)."""
    s = np.zeros((128, 24, 24), np.float32)
    for w in range(24):
        s[:, w, w] = 1.0
    return s.reshape(128, 24 * 24).astype(BF)


def _onesm():
    """[24, 24, 128] bf16: ONESM[r, w, :] = (r == w)."""
    s = np.zeros((24, 24, 128), np.float32)
    for w in range(24):
        s[w, w, :] = 1.0
    return s.reshape(24, 24 * 128).astype(BF)




# two batches: batch b covers graphs {2b, 2b+1}; 8 dedicated + 4 shared each.
# slot s order: [b0: ded g0k0..g1k3, sh j0g0, j0g1, j1g0, j1g1] then batch 1.
def _slots():
    out = []   # per slot: (graph, wi, b1idx)  wi: index into W2Psel/b2Psel
    nded = 0
    for b in range(2):
        for g in (2 * b, 2 * b + 1):
            for k in range(TOPK):
                out.append((g, nded, nded))
                nded += 1
        for j in range(KS):
            for g in (2 * b, 2 * b + 1):
                out.append((g, NSLOT + j, -1 - j))
    return out


SLOTS = _slots()
DED_GK = []   # (graph, k) in packed ded order
for b in range(2):
    for g in (2 * b, 2 * b + 1):
        for k in range(TOPK):
            DED_GK.append((g, k))

# ------------------------------------------------------------- build kernel1

DEBUG_K1 = False


def _build_k1(CW):
    ntot = int(sum(CW))
    nc = bacc.Bacc("TRN2", target_bir_lowering=False, debug=False,
                   num_devices=NCORE)

    def din(name, shape, dt=F32):
        return nc.dram_tensor(name, shape, dt, kind="ExternalInput")

    ecf_i = din("ecf", [128, ntot * CF1], BF16)
    oea_i = din("oea", [128, ntot * 128], BF16)
    Wca_i = din("Wca", [CF1, D], BF16)
    Wv_i = din("Wv", [VF, D])
    bv_i = din("bv_col", [D, 1])
    vfT_i = din("vfeatT", [VF, NC_NODES])
    We_i = din("We_col", [D, 1])
    lng_i = din("lng_col", [D, 1])
    lnb_i = din("lnb_col", [D, 1])
    P_i = din("P_bf", [128, 128], BF16)
    WqT_i = din("WqT", [TD, D])
    tokKT_i = din("tokKT", [TD, T])
    bq_i = din("bq_col", [TD, 1])
    tokV_i = din("tokV", [T, TD], BF16)
    Wg_i = din("Wg_r", [D, 2, NE])
    bg_i = din("bg_col", [NE, 1])
    eb_i = din("eb_col", [NE, 1])
    al_i = din("alpha11", [1, 1], BF16)
    sel24_i = din("sel24", [128, 24 * 24], BF16)
    onesm_i = din("onesm", [24, 24 * 128], BF16)
    padc4_i = din("padc4", [128, GPC])
    invc4_i = din("invc4", [128, GPC])
    negpadc_i = din("negpadc", [1, GPC], BF16)
    W2a_i = din("W2all", [D, NSLOT + KS, 4, 128], BF16)
    b2a_i = din("b2allT", [D, NSLOT + KS], BF16)

    vembT_o = nc.dram_tensor("vembT", [D, NC_NODES], BF16, kind="ExternalOutput")
    exlg_o = nc.dram_tensor("explogT", [NE, GPC], F32, kind="ExternalOutput")
    W2P_o = nc.dram_tensor("W2Pall", [D, (NSLOT + KS) * 4 * 128], BF16,
                           kind="ExternalOutput")
    b2P_o = nc.dram_tensor("b2Pall", [D, NSLOT + KS], F32,
                           kind="ExternalOutput")

    offs = np.concatenate([[0], np.cumsum(CW)]).astype(int)
    goffs = [int(offs[4 * g]) for g in range(NGRP + 1)]   # tile offsets per group

    with tile.TileContext(nc) as tc:
        with (
            tc.tile_pool(name="cp", bufs=1) as cp,
            tc.tile_pool(name="oh", bufs=2) as ohp,
            tc.tile_pool(name="wk", bufs=3) as wk,
            tc.tile_pool(name="sm", bufs=4) as smp,
            tc.tile_pool(name="ps", bufs=1, space="PSUM") as ps,
        ):
            PS_BUFS = {"g1": 2, "mm": 3, "pa": 2}
            _ld = [0]
            def load(ap_dram, shape, dt=F32):
                _ld[0] += 1
                t_ = cp.tile(shape, dt, tag=f"cst{_ld[0]}", name=f"cst{_ld[0]}")
                src_ap = ap_dram[:]
                if dt == F32R:
                    src_ap = src_ap.bitcast(F32R)
                nc.sync.dma_start(t_[:], src_ap)
                return t_

            ecf_s = load(ecf_i, [128, ntot * CF1], BF16)
            # group 0/1 one-hot DMAs first: they head the critical path
            oea_pre = []
            for _g in range(2):
                gt0, gt1 = goffs[_g], goffs[_g + 1]
                _t = ohp.tile([128, 32 * 128], BF16, tag="oea", name="oeaw")
                nc.sync.dma_start(_t[:, :(gt1 - gt0) * 128],
                                  oea_i[:, gt0 * 128:gt1 * 128])
                oea_pre.append(_t)
            Wca_s = load(Wca_i, [CF1, D], BF16)
            Wv_s = load(Wv_i, [VF, D], F32R)
            bv_s = load(bv_i, [D, 1])
            vfT_s = load(vfT_i, [VF, NC_NODES], F32R)
            We_s = load(We_i, [D, 1])
            lng_s = load(lng_i, [D, 1])
            lnb_s = load(lnb_i, [D, 1])
            P_s = load(P_i, [128, 128], BF16)
            WqT_s = load(WqT_i, [TD, D], F32R)
            tKT_s = load(tokKT_i, [TD, T], F32R)
            bq_s = load(bq_i, [TD, 1], F32R)
            tV_s = load(tokV_i, [T, TD], BF16)
            Wg_s = load(Wg_i, [D, 2, NE], F32R)
            bg_s = load(bg_i, [NE, 1])
            eb_s = load(eb_i, [NE, 1])
            al_s = load(al_i, [1, 1], BF16)
            sel24 = load(sel24_i, [128, 24, 24], BF16)
            onesm = load(onesm_i, [24, 24, 128], BF16)
            padc4 = load(padc4_i, [128, GPC])
            invc4 = load(invc4_i, [128, GPC])
            negpadc = load(negpadc_i, [1, GPC], BF16)

            onesr_bf = cp.tile([1, 128], BF16, name="onesr_bf")
            nc.vector.memset(onesr_bf[:], 1.0)
            onesc_bf = cp.tile([128, 1], BF16, name="onesc_bf")
            nc.vector.memset(onesc_bf[:], 1.0)
            eps24 = cp.tile([24, 1], F32, name="eps24")
            nc.vector.memset(eps24[:], LN_EPS)

            # persistent big tiles
            c_all = cp.tile([128, NGRP, 4, 128], F32, name="c_all")
            v0b_all = cp.tile([128, NGRP, 512], F32, name="v0b_all")
            vembT_s = cp.tile([128, NWIN, 128], BF16, name="vembT_s")
            wsum = cp.tile([128, NWIN], F32, name="wsum")
            varsb = cp.tile([24, NGRP, 128], F32, name="varsb")
            rstd24 = cp.tile([24, NGRP, 128], BF16, name="rstd24")
            Wp_s = cp.tile([D, T], BF16, name="Wp_s")       # Wq @ tokK^T
            bqK_s = cp.tile([1, T], BF16, name="bqK_s")

            # ---- prologue: W' = Wq @ tokK^T  [D, T]; bqK = bq^T tokK^T
            pWp = ps.tile([128, 512], F32, tag="mm", name="pWp",
                          bufs=PS_BUFS["mm"])
            nc.tensor.matmul(pWp[:, :T], WqT_s[:], tKT_s[:], start=True, stop=True)
            nc.vector.tensor_copy(Wp_s[:], pWp[:, :T])
            pbq = ps.tile([NE, 512], F32, tag="g1", name="pbq",
                          bufs=PS_BUFS["g1"])
            nc.tensor.matmul(pbq[:1, :T], bq_s[:], tKT_s[:], start=True, stop=True)
            nc.vector.tensor_copy(bqK_s[:], pbq[:1, :T])

            # ---- v0 for all groups up front (independent of edges)
            for grp in range(NGRP):
                pv0 = ps.tile([128, 512], F32, tag="mm", name="pv0",
                              bufs=PS_BUFS["mm"])
                nc.tensor.matmul(pv0[:], Wv_s[:],
                                 vfT_s[:, grp * 512:(grp + 1) * 512],
                                 start=True, stop=True)
                nc.vector.tensor_scalar(v0b_all[:, grp, :], pv0[:], bv_s[:],
                                        None, ALU.add)

            # ---- pad-column head: x=relu(bv); c=P x; var -> varsb[0, 5, 0]
            z0 = smp.tile([128, 1], F32, tag="pad", name="z0")
            nc.vector.memset(z0[:], 0.0)
            xp = smp.tile([128, 1], BF16, tag="padb", name="xp")
            nc.scalar.activation(xp[:], z0[:], AF.Relu, bias=bv_s[:])
            pcp = ps.tile([128, 512], F32, tag="mm", name="pcp",
                          bufs=PS_BUFS["mm"])
            nc.tensor.matmul(pcp[:, :1], P_s[:], xp[:], start=True, stop=True)
            cgp = smp.tile([128, 1], F32, tag="pad", name="cgp")
            nc.vector.tensor_scalar(cgp[:], pcp[:, :1], lng_s[:], None, ALU.mult)
            sqp = smp.tile([128, 1], BF16, tag="padb", name="sqp")
            nc.vector.tensor_tensor(sqp[:], cgp[:], cgp[:], ALU.mult)
            pvp = ps.tile([NE, 512], F32, tag="g1", name="pvp",
                          bufs=PS_BUFS["g1"])
            nc.tensor.matmul(pvp[:1, :1], onesc_bf[:], sqp[:], start=True, stop=True)
            nc.vector.tensor_copy(varsb[0:1, NGRP - 1:NGRP, 0:1], pvp[:1, :1])

            # ---- phase 1, software pipelined: G1(g) | midA(g-1) | midB(g-2)
            def midA(grp):
                pT1 = ps.tile([128, 512], F32, tag="mm", name="pT1",
                              bufs=PS_BUFS["mm"])
                nc.tensor.matmul(pT1[:], Wca_s[:], G1t[grp][:],
                                 start=True, stop=True)
                s_sb = wk.tile([128, 512], F32, tag="s", name="s_sb")
                nc.vector.scalar_tensor_tensor(
                    s_sb[:], pT1[:], We_s[:], v0b_all[:, grp, :],
                    ALU.mult, ALU.add)
                x_bf = wk.tile([128, 512], BF16, tag="x", name="x_bf")
                nc.scalar.activation(x_bf[:], s_sb[:], AF.Relu)
                pc = ps.tile([128, 512], F32, tag="mm", name="pc",
                             bufs=PS_BUFS["mm"])
                nc.tensor.matmul(pc[:], P_s[:], x_bf[:], start=True, stop=True)
                nc.vector.tensor_scalar(
                    c_all[:, grp, :, :], pc[:], lng_s[:], None, ALU.mult)
                sqt = wk.tile([128, 4, 128], BF16, tag="sq", name="sqt")
                nc.vector.tensor_tensor(sqt[:], c_all[:, grp, :, :],
                                        c_all[:, grp, :, :], ALU.mult)
                sq_t[grp] = sqt

            def midB(grp):
                pvarg = ps.tile([24, 128], F32, tag="g1", name="pvarg",
                                bufs=PS_BUFS["g1"])
                for wi in range(4):
                    w = grp * 4 + wi
                    nc.tensor.matmul(pvarg[:], sel24[:, w, :],
                                     sq_t[grp][:, wi, :],
                                     start=(wi == 0), stop=(wi == 3))
                nc.vector.tensor_copy(varsb[:, grp, :], pvarg[:])

            G1t = [None] * NGRP
            sq_t = [None] * NGRP
            for grp in range(NGRP):
                gt0, gt1 = goffs[grp], goffs[grp + 1]
                nt = gt1 - gt0
                if grp < 2:
                    oeaw = oea_pre[grp]
                else:
                    oeaw = ohp.tile([128, 32 * 128], BF16, tag="oea",
                                    name="oeaw")
                    nc.sync.dma_start(oeaw[:, :nt * 128],
                                      oea_i[:, gt0 * 128:gt1 * 128])
                pG1 = ps.tile([5, 512], F32, tag="g1", name="pG1",
                              bufs=PS_BUFS["g1"])
                for wi in range(4):
                    w = grp * 4 + wi
                    for t_ in range(int(CW[w])):
                        gt = int(offs[w]) + t_
                        lt = gt - gt0
                        nc.tensor.matmul(
                            pG1[:CF1, wi * 128:(wi + 1) * 128],
                            ecf_s[:, gt * CF1:(gt + 1) * CF1],
                            oeaw[:, lt * 128:(lt + 1) * 128],
                            start=(t_ == 0), stop=(t_ == int(CW[w]) - 1))
                G1sb = wk.tile([CF1, 512], BF16, tag="g1sb", bufs=2, name="G1sb")
                nc.vector.tensor_copy(G1sb[:], pG1[:CF1, :])
                G1t[grp] = G1sb
                if grp >= 1:
                    midA(grp - 1)
                if grp >= 2:
                    midB(grp - 2)
            midA(NGRP - 1)
            midB(NGRP - 2)
            midB(NGRP - 1)

            # W2 fold inputs: issue DMA now so it rides behind the oea loads
            W2a_s = cp.tile([D, NSLOT + KS, 4, 128], BF16, name="W2a_s")
            nc.sync.dma_start(W2a_s[:], W2a_i[:])
            b2a_s = cp.tile([D, NSLOT + KS], BF16, name="b2a_s")
            nc.sync.dma_start(b2a_s[:], b2a_i[:])

            # ---- rstd for all windows (incl pad at [0, NGRP-1, 0])
            lnv = wk.tile([24, NGRP, 128], F32, tag="lnv", bufs=1, name="lnv")
            nc.scalar.activation(lnv[:], varsb[:], AF.Ln,
                                 bias=eps24[:], scale=1.0 / D)
            nc.scalar.activation(rstd24[:], lnv[:], AF.Exp, scale=-0.5)

            # ---- pad-column tail (uses batched pad rstd)
            pbb = ps.tile([128, 512], F32, tag="mm", name="pbb",
                          bufs=PS_BUFS["mm"])
            nc.tensor.matmul(pbb[:, :1], onesr_bf[:],
                             rstd24[0:1, NGRP - 1, 0:1], start=True, stop=True)
            up = smp.tile([128, 1], F32, tag="pad", name="up")
            nc.vector.tensor_tensor(up[:], cgp[:], pbb[:, :1], ALU.mult)
            vp = smp.tile([128, 1], BF16, tag="padb", name="vp")
            nc.vector.tensor_scalar(vp[:], up[:], lnb_s[:], None, ALU.add)
            pscp = ps.tile([NE, 512], F32, tag="g1", name="pscp",
                           bufs=PS_BUFS["g1"])
            nc.tensor.matmul(pscp[:1, :T], vp[:], Wp_s[:], start=True, stop=False)
            nc.tensor.matmul(pscp[:1, :T], onesr_bf[:, :1], bqK_s[:],
                             start=False, stop=True)
            exps = smp.tile([1, T], F32, tag="padr", name="exps")
            nc.scalar.activation(exps[:], pscp[:1, :T], AF.Exp, scale=ISQ_TD)
            smsum = smp.tile([1, 1], F32, tag="pads", name="smsum")
            nc.vector.tensor_reduce(smsum[:], exps[:], AX.X, ALU.add)
            rcp = smp.tile([1, 1], F32, tag="pads", name="rcp")
            nc.vector.reciprocal(rcp[:], smsum[:])
            wtsp = smp.tile([1, T], BF16, tag="padr", name="wtsp")
            nc.vector.tensor_scalar(wtsp[:], exps[:], rcp[:], None, ALU.mult)

            # ---- phase 2 + struct scores, software pipelined per group
            R = ps.tile([64, 8], F32, tag="g1", name="R", bufs=PS_BUFS["g1"])

            def rowsums(grp):
                for wi in range(4):
                    w = grp * 4 + wi
                    g, j = w // WPG, w % WPG
                    nc.tensor.matmul(R[:T, g:g + 1], wts_t[grp][:, wi, :],
                                     onesc_bf[:], start=(j == 0),
                                     stop=(j == WPG - 1))

            wts_t = [None] * NGRP
            for grp in range(NGRP):
                pA = ps.tile([128, 4, 128], F32, tag="pa", name="pA",
                             bufs=PS_BUFS["pa"])
                for wi in range(4):
                    w = grp * 4 + wi
                    nc.tensor.matmul(pA[:, wi, :], onesm[:, w, :],
                                     rstd24[:, grp, :], start=True, stop=True)
                u_sb = wk.tile([128, 4, 128], F32, tag="u", name="u_sb")
                nc.vector.tensor_tensor(u_sb[:], c_all[:, grp, :, :], pA[:],
                                        ALU.mult)
                nc.scalar.activation(vembT_s[:, 4 * grp:4 * grp + 4, :],
                                      u_sb[:], AF.Identity, bias=lnb_s[:])
                nc.vector.tensor_reduce(wsum[:, 4 * grp:4 * grp + 4],
                                        u_sb[:], AX.X, ALU.add)
                psc = ps.tile([128, 4, 64], F32, tag="pa", name="psc",
                              bufs=PS_BUFS["pa"])
                for wi in range(4):
                    w = grp * 4 + wi
                    nc.tensor.matmul(psc[:, wi, :], vembT_s[:, w, :], Wp_s[:],
                                     start=True, stop=False)
                    nc.tensor.matmul(psc[:, wi, :], onesr_bf[:], bqK_s[:],
                                     start=False, stop=True)
                ex = wk.tile([128, 4, 64], BF16, tag="ex", bufs=2, name="ex")
                nc.scalar.activation(ex[:], psc[:], AF.Exp, scale=ISQ_TD)
                sme = smp.tile([128, 4], F32, tag="sme", bufs=3, name="sme")
                nc.vector.tensor_reduce(sme[:], ex[:], AX.X, ALU.add)
                rce = smp.tile([128, 4], F32, tag="rce", bufs=3, name="rce")
                nc.vector.reciprocal(rce[:], sme[:])
                wts = wk.tile([128, 4, 64], BF16, tag="wts", bufs=3, name="wts")
                for wi in range(4):
                    nc.vector.tensor_scalar(wts[:, wi, :], ex[:, wi, :],
                                            rce[:, wi:wi + 1], None, ALU.mult)
                wts_t[grp] = wts
                if grp >= 1:
                    rowsums(grp - 1)
            rowsums(NGRP - 1)
            nc.tensor.matmul(R[:T, GPC:2 * GPC], wtsp[:], negpadc[:],
                             start=True, stop=True)

            nc.sync.dma_start(vembT_o[:], vembT_s[:])

            # ---- struct pooling
            Rsb = smp.tile([64, 2 * GPC], F32, tag="Rsb", bufs=1, name="Rsb")
            nc.vector.tensor_copy(Rsb[:], R[:T, :2 * GPC])
            Rc = smp.tile([64, GPC], BF16, tag="Rc", bufs=1, name="Rc")
            nc.vector.tensor_tensor(Rc[:], Rsb[:, :GPC], Rsb[:, GPC:2 * GPC],
                                    ALU.add)
            pstr = ps.tile([128, 512], F32, tag="mm", name="pstr",
                           bufs=PS_BUFS["mm"])
            nc.tensor.matmul(pstr[:, :GPC], tV_s[:], Rc[:], start=True, stop=True)
            strT = smp.tile([128, GPC], F32R, tag="strT", bufs=1, name="strT")
            with nc.allow_low_precision(reason="gating rhs f32r"):
                nc.vector.tensor_tensor(strT[:], pstr[:, :GPC], invc4[:],
                                        ALU.mult)

            # ---- graph embedding pooling with pad correction
            gsum = smp.tile([128, GPC], F32, tag="gsum", bufs=1, name="gsum")
            for g in range(GPC):
                nc.vector.tensor_reduce(gsum[:, g:g + 1],
                                        wsum[:, g * WPG:(g + 1) * WPG],
                                        AX.X, ALU.add)
            t3 = smp.tile([128, GPC], F32, tag="t3", bufs=1, name="t3")
            nc.vector.tensor_scalar(t3[:], padc4[:], up[:], None, ALU.mult)
            t4 = smp.tile([128, GPC], F32, tag="t4", bufs=1, name="t4")
            nc.vector.tensor_tensor(t4[:], gsum[:], t3[:], ALU.subtract)
            t5 = smp.tile([128, GPC], F32, tag="t5", bufs=1, name="t5")
            nc.vector.tensor_tensor(t5[:], t4[:], invc4[:], ALU.mult)
            gembT = smp.tile([128, GPC], F32R, tag="gembT", bufs=1, name="gembT")
            with nc.allow_low_precision(reason="gating rhs f32r"):
                nc.vector.tensor_scalar(gembT[:], t5[:], lnb_s[:], None, ALU.add)

            # ---- gating logits -> exp(logits)
            pl = ps.tile([NE, 512], F32, tag="g1", name="pl", bufs=PS_BUFS["g1"])
            nc.tensor.matmul(pl[:, :GPC], Wg_s[:, 0, :], gembT[:],
                             start=True, stop=False)
            nc.tensor.matmul(pl[:, :GPC], Wg_s[:, 1, :], strT[:],
                             start=False, stop=True)
            pa_ = ps.tile([128, 512], F32, tag="mm", name="pa_",
                          bufs=PS_BUFS["mm"])
            nc.tensor.matmul(pa_[:NE, :1], onesr_bf[:, :NE], al_s[:],
                             start=True, stop=True)
            acol = smp.tile([NE, 1], F32, tag="acol", bufs=1, name="acol")
            nc.vector.tensor_copy(acol[:], pa_[:NE, :1])
            lg1 = smp.tile([NE, GPC], F32, tag="lg1", bufs=1, name="lg1")
            nc.vector.tensor_scalar(lg1[:], pl[:, :GPC], bg_s[:], None, ALU.add)
            lg2 = smp.tile([NE, GPC], F32, tag="lg2", bufs=1, name="lg2")
            nc.vector.tensor_scalar(lg2[:], lg1[:], acol[:], 1.0 / TEMP,
                                    ALU.mult, ALU.mult)
            lg3 = smp.tile([NE, GPC], F32, tag="lg3", bufs=1, name="lg3")
            nc.vector.tensor_scalar(lg3[:], lg2[:], eb_s[:], None, ALU.add)
            exlg = smp.tile([NE, GPC], F32, tag="exlg", bufs=1, name="exlg")
            nc.scalar.activation(exlg[:], lg3[:], AF.Exp)
            nc.sync.dma_start(exlg_o[:], exlg[:])

            # ---- W2 fold for all experts: W2P = (W2_chunk @ P), h-major
            W2P = cp.tile([128, NSLOT + KS, 4, 128], BF16, name="W2P")
            for s in range(NSLOT + KS):
                pw = ps.tile([128, 512], F32, tag="mm", name="pw",
                             bufs=PS_BUFS["mm"])
                for c in range(4):
                    nc.tensor.matmul(pw[:, c * 128:(c + 1) * 128],
                                     W2a_s[:, s, c, :], P_s[:],
                                     start=True, stop=True)
                nc.scalar.copy(W2P[:, s, :, :], pw[:])
            nc.sync.dma_start(W2P_o[:], W2P[:])
            pb2 = ps.tile([128, 512], F32, tag="mm", name="pb2",
                          bufs=PS_BUFS["mm"])
            nc.tensor.matmul(pb2[:, :NSLOT + KS], P_s[:], b2a_s[:],
                             start=True, stop=True)
            b2P = cp.tile([D, NSLOT + KS], F32, name="b2P")
            nc.vector.tensor_copy(b2P[:], pb2[:, :NSLOT + KS])
            nc.sync.dma_start(b2P_o[:], b2P[:])

    nc.compile()
    return nc


# ------------------------------------------------------------- build kernel2

def _build_k2():
    nc = bacc.Bacc("TRN2", target_bir_lowering=False, debug=False,
                   num_devices=NCORE)

    def din(name, shape, dt=F32):
        return nc.dram_tensor(name, shape, dt, kind="ExternalInput")

    vembT_i = din("vembT_bf", [D, NC_NODES], BF16)
    explog_i = din("explog_nm", [GPC, NE])
    mask_i = din("mask_nm", [GPC, NE])
    Esel_i = din("Esel24", [24, NE])
    Gsel_i = din("Gsel24", [GPC, 24])
    sh05_i = din("sh05", [24, 1])
    W1sel_i = din("W1sel", [D, NSLOT, 4 * D], BF16)
    sW1_i = din("sW1T", [D, KS, 4 * D], BF16)
    b1selT_i = din("b1selT", [128, NSLOT * 4])
    sb1T_i = din("sb1T", [128, KS * 4])
    W2P_i = din("W2Psel", [D, NSLOT + KS, 4, 128], BF16)
    b2P_i = din("b2Psel", [D, NSLOT + KS])
    wgm_i = din("wgm", [12, NCH * 128], BF16)
    sel24_i = din("sel24", [128, 24 * 24], BF16)
    shifts_i = din("shifts", [24, 2 * 12], BF16)
    bb24_i = din("bb24", [24, D], BF16)
    gmask_i = din("gmask24", [24, GPC])
    hW1_i = din("hW1", [D, D], BF16)
    hb1_i = din("hb1_col", [D, 1])
    hW2_i = din("hW2col", [D, 1], BF16)
    hb2_i = din("hb2", [1, 1])

    out_o = nc.dram_tensor("out_row", [1, NC_NODES], F32, kind="ExternalOutput")

    with tile.TileContext(nc) as tc:
        with (
            tc.tile_pool(name="cp", bufs=1) as cp,
            tc.tile_pool(name="wk", bufs=3) as wk,
            tc.tile_pool(name="sm", bufs=4) as smp,
            tc.tile_pool(name="ps", bufs=1, space="PSUM") as ps,
        ):
            PS_BUFS = {"ph": 3, "pc": 3, "var": 1}
            _ld = [0]
            def load(ap_dram, shape, dt=F32):
                _ld[0] += 1
                t_ = cp.tile(shape, dt, tag=f"cst{_ld[0]}", name=f"cst{_ld[0]}")
                src_ap = ap_dram[:]
                if dt == F32R:
                    src_ap = src_ap.bitcast(F32R)
                nc.sync.dma_start(t_[:], src_ap)
                return t_

            # batch-0 slot data first in the DMA queue
            vembT = cp.tile([D, NC_NODES], BF16, tag="cvembT", name="vembT")
            nc.sync.dma_start(vembT[:, :NC_NODES // 2],
                              vembT_i[:, :NC_NODES // 2])
            W1 = cp.tile([D, NSLOT, 4 * D], BF16, tag="cW1", name="W1")
            nc.sync.dma_start(W1[:, :8, :], W1sel_i[:, :8, :])
            b1T = load(b1selT_i, [128, NSLOT * 4])
            sb1T = load(sb1T_i, [128, KS * 4])
            b2P = load(b2P_i, [D, NSLOT + KS])
            sW1 = load(sW1_i, [D, KS, 4 * D], BF16)
            W2P = cp.tile([D, NSLOT + KS, 4, 128], BF16, tag="cW2P",
                          name="W2P")
            nc.sync.dma_start(W2P[:, :8, :, :], W2P_i[:, :8, :, :])
            nc.sync.dma_start(W2P[:, NSLOT:, :, :], W2P_i[:, NSLOT:, :, :])
            wgm = load(wgm_i, [12, NCH, 128], BF16)
            shifts = load(shifts_i, [24, 2, 12], BF16)
            sel24 = load(sel24_i, [128, 24, 24], BF16)
            exlg = load(explog_i, [GPC, NE])
            msk = load(mask_i, [GPC, NE])
            Esel = load(Esel_i, [24, NE])
            Gsel = load(Gsel_i, [GPC, 24], F32R)
            sh05 = load(sh05_i, [24, 1])
            bb24 = load(bb24_i, [24, D], BF16)
            gmask = load(gmask_i, [24, GPC])
            hW1 = load(hW1_i, [D, D], BF16)
            hb1 = load(hb1_i, [D, 1])
            hW2 = load(hW2_i, [D, 1], BF16)
            hb2 = load(hb2_i, [1, 1])
            # batch-1 slot data at the tail of the DMA queue
            nc.sync.dma_start(vembT[:, NC_NODES // 2:],
                              vembT_i[:, NC_NODES // 2:])
            nc.sync.dma_start(W1[:, 8:, :], W1sel_i[:, 8:, :])
            nc.sync.dma_start(W2P[:, 8:NSLOT, :, :], W2P_i[:, 8:NSLOT, :, :])

            eps24 = cp.tile([24, 1], F32, name="eps24")
            nc.vector.memset(eps24[:], LN_EPS)

            acc = cp.tile([D, NC_NODES], F32, name="acc")
            cbS = cp.tile([128, NCH, 2, HF], BF16, name="cbS")
            out_sb = cp.tile([1, NC_NODES], F32, name="out_sb")

            # ---- route weights on device (exp(logits) comes from k1)
            sme = smp.tile([GPC, 1], F32, tag="sme", bufs=1, name="sme")
            nc.vector.tensor_reduce(sme[:], exlg[:], AX.X, ALU.add)
            rce = smp.tile([GPC, 1], F32, tag="rce", bufs=1, name="rce")
            nc.vector.reciprocal(rce[:], sme[:])
            w_sm = smp.tile([GPC, NE], F32, tag="w_sm", bufs=1, name="w_sm")
            nc.vector.tensor_scalar(w_sm[:], exlg[:], rce[:], None, ALU.mult)
            wm = smp.tile([GPC, NE], F32, tag="wm", bufs=1, name="wm")
            nc.vector.tensor_tensor(wm[:], w_sm[:], msk[:], ALU.mult)
            s2_ = smp.tile([GPC, 1], F32, tag="s2_", bufs=1, name="s2_")
            nc.vector.tensor_reduce(s2_[:], wm[:], AX.X, ALU.add)
            s2e = smp.tile([GPC, 1], F32, tag="s2e", bufs=1, name="s2e")
            nc.gpsimd.tensor_scalar(s2e[:], s2_[:], 1e-12, None, ALU.add)
            rc2 = smp.tile([GPC, 1], F32, tag="rc2", bufs=1, name="rc2")
            nc.vector.reciprocal(rc2[:], s2e[:])
            route = smp.tile([GPC, NE], F32, tag="route", bufs=1, name="route")
            nc.vector.tensor_scalar(route[:], wm[:], rc2[:], None, ALU.mult)
            route_r = smp.tile([GPC, NE], F32R, tag="route_r", bufs=1,
                               name="route_r")
            with nc.allow_low_precision(reason="route f32r view"):
                nc.vector.tensor_copy(route_r[:], route[:])

            pR2 = ps.tile([128, 512], F32, tag="pc", name="pR2",
                          bufs=PS_BUFS["pc"])
            nc.tensor.matmul(pR2[:24, :NE], Gsel[:], route_r[:],
                             start=True, stop=True)
            r2e = smp.tile([24, NE], F32, tag="r2e", bufs=1, name="r2e")
            nc.vector.tensor_tensor(r2e[:], pR2[:24, :NE], Esel[:], ALU.mult)
            wc24 = smp.tile([24, 1], F32, tag="wc24", bufs=1, name="wc24")
            nc.vector.tensor_reduce(wc24[:], r2e[:], AX.X, ALU.add)
            wcol24 = cp.tile([24, 1], F32, name="wcol24")
            nc.vector.tensor_tensor(wcol24[:], wc24[:], sh05[:], ALU.add)
            wcol24_bf = cp.tile([24, 1], BF16, name="wcol24_bf")
            nc.vector.tensor_copy(wcol24_bf[:], wcol24[:])
            wcolb = []
            for b in range(2):
                pwc = ps.tile([128, 512], F32, tag="pc", name="pwc",
                              bufs=PS_BUFS["pc"])
                nc.tensor.matmul(pwc[:12, :1], shifts[:, b, :], wcol24_bf[:],
                                 start=True, stop=True)
                wcb = cp.tile([12, 1], F32, name=f"wcb{b}")
                nc.vector.tensor_copy(wcb[:], pwc[:12, :1])
                wcolb.append(wcb)

            # per-graph LN bias columns: biasg = bb24^T @ (gmask * wcol24)
            wsel24 = smp.tile([24, GPC], BF16, tag="wsel", bufs=1,
                              name="wsel24")
            nc.vector.tensor_scalar(wsel24[:], gmask[:], wcol24[:], None,
                                    ALU.mult)
            pbg = ps.tile([128, 512], F32, tag="pc", name="pbg",
                          bufs=PS_BUFS["pc"])
            nc.tensor.matmul(pbg[:, :GPC], bb24[:], wsel24[:],
                             start=True, stop=True)
            biasg = cp.tile([D, GPC], F32, name="biasg")
            nc.vector.tensor_copy(biasg[:], pbg[:, :GPC])

            # ---- expert pipeline, two batches of 12 slots; pass B / head of
            # batch b overlaps pass A of batch b+1
            pvar = ps.tile([12, 2, 512], F32, tag="var", name="pvar",
                           bufs=PS_BUFS["var"])
            sq_t = [None] * NCH
            rstdw_t = [None, None]
            first = set()

            def emit_front(s, local, last_local):
                g, wi, b1i = SLOTS[s]
                off = g * PAD_G
                if b1i >= 0:
                    W1t = W1[:, b1i, :]
                    b1c = b1T[:, b1i * 4:(b1i + 1) * 4]
                else:
                    j = -1 - b1i
                    W1t = sW1[:, j, :]
                    b1c = sb1T[:, j * 4:(j + 1) * 4]
                hTns = []
                for h in range(2):
                    for c in range(4):
                        ph = ps.tile([128, HF], F32, tag="ph", name="ph",
                                     bufs=PS_BUFS["ph"])
                        nc.tensor.matmul(
                            ph[:], W1t[:, c * 128:(c + 1) * 128],
                            vembT[:, off + h * HF:off + (h + 1) * HF],
                            start=True, stop=True)
                        hTn = wk.tile([128, HF], BF16, tag="hTn", bufs=10,
                                      name="hTn")
                        nc.scalar.activation(hTn[:], ph[:], AF.Gelu,
                                             bias=b1c[:, c:c + 1])
                        hTns.append(hTn)
                if local >= 1:
                    emit_var(s - 1, local - 1, last_local)
                for h in range(2):
                    pc_ = ps.tile([128, HF], F32, tag="pc", name="pc_",
                                  bufs=PS_BUFS["pc"])
                    for c in range(4):
                        nc.tensor.matmul(pc_[:], W2P[:, wi, c, :],
                                         hTns[h * 4 + c][:],
                                         start=(c == 0), stop=(c == 3))
                    nc.vector.tensor_scalar(cbS[:, s, h, :], pc_[:],
                                            b2P[:, wi:wi + 1], None, ALU.add)
                sqt = wk.tile([128, 2, HF], BF16, tag="sq", bufs=3, name="sqt")
                nc.vector.tensor_tensor(sqt[:], cbS[:, s, :, :],
                                        cbS[:, s, :, :], ALU.mult)
                sq_t[s] = sqt

            def emit_var(s, local, last_local):
                for h in range(2):
                    nc.tensor.matmul(pvar[:, h, :HF], sel24[:, local, :12],
                                     sq_t[s][:, h, :],
                                     start=(local == 0),
                                     stop=(local == last_local))

            def emit_rstd(b):
                lnv = wk.tile([12, 2, HF], F32, tag="lnv", bufs=2, name="lnv")
                nc.scalar.activation(lnv[:], pvar[:, :, :HF],
                                     AF.Ln, bias=eps24[:12, :],
                                     scale=1.0 / D)
                rstd = wk.tile([12, 2, HF], BF16, tag="rstd", bufs=2,
                               name="rstd")
                nc.scalar.activation(rstd[:], lnv[:], AF.Exp, scale=-0.5)
                rstdw = wk.tile([12, 2, HF], BF16, tag="rstdw", bufs=2,
                                name="rstdw")
                nc.vector.tensor_scalar(rstdw[:], rstd[:],
                                        wcolb[b][:], None, ALU.mult)
                rstdw_t[b] = rstdw

            def passB_order(b):
                base = 12 * b
                order = []
                for k in range(TOPK):
                    for gl in range(2):
                        order.append(base + gl * TOPK + k)
                for j in range(KS):
                    for gl in range(2):
                        order.append(base + 8 + j * 2 + gl)
                return order

            def emit_passB(b, order):
                for s in order:
                    g, _, _ = SLOTS[s]
                    off = g * PAD_G
                    for h in range(2):
                        pA = ps.tile([128, HF], F32, tag="ph", name="pA",
                                     bufs=PS_BUFS["ph"])
                        nc.tensor.matmul(pA[:], wgm[:, s, :],
                                         rstdw_t[b][:, h, :],
                                         start=True, stop=True)
                        u = wk.tile([128, HF], F32, tag="u", bufs=4, name="u")
                        nc.vector.tensor_tensor(u[:], cbS[:, s, h, :], pA[:],
                                                ALU.mult)
                        asl = acc[:, off + h * HF:off + (h + 1) * HF]
                        if (off, h) not in first:
                            first.add((off, h))
                            nc.vector.tensor_tensor(
                                asl, u[:],
                                vembT[:, off + h * HF:off + (h + 1) * HF],
                                ALU.add)
                        else:
                            nc.vector.tensor_tensor(asl, asl, u[:], ALU.add)

            def emit_head(b):
                for g in (2 * b, 2 * b + 1):
                    off = g * PAD_G
                    asl = acc[:, off:off + PAD_G]
                    nc.vector.tensor_scalar(asl, asl, biasg[:, g:g + 1], None,
                                            ALU.add)
                    acc_bf = wk.tile([128, PAD_G], BF16, tag="accbf", bufs=2,
                                     name="acc_bf")
                    nc.vector.tensor_copy(acc_bf[:], asl)
                    for h in range(2):
                        pr = ps.tile([128, HF], F32, tag="ph", name="pr",
                                     bufs=PS_BUFS["ph"])
                        nc.tensor.matmul(pr[:], hW1[:],
                                         acc_bf[:, h * HF:(h + 1) * HF],
                                         start=True, stop=True)
                        r_bf = wk.tile([128, HF], BF16, tag="rbf", bufs=3,
                                       name="r_bf")
                        nc.scalar.activation(r_bf[:], pr[:], AF.Relu,
                                             bias=hb1[:])
                        po = ps.tile([128, HF], F32, tag="pc", name="po",
                                     bufs=PS_BUFS["pc"])
                        nc.tensor.matmul(po[:1, :], hW2[:], r_bf[:],
                                         start=True, stop=True)
                        nc.vector.tensor_scalar(
                            out_sb[:, off + h * HF:off + (h + 1) * HF],
                            po[:1, :], hb2[:], None, ALU.add)

            # batch 0 fronts
            for local in range(12):
                emit_front(local, local, 11)
            emit_var(11, 11, 11)
            emit_rstd(0)
            # batch 1 fronts, interleaved slot-by-slot with batch 0's pass B
            ord0 = passB_order(0)
            for local in range(12):
                emit_front(12 + local, local, 11)
                emit_passB(0, [ord0[local]])
            emit_var(23, 11, 11)
            emit_head(0)
            emit_rstd(1)
            emit_passB(1, passB_order(1))
            emit_head(1)

            nc.sync.dma_start(out_o[:], out_sb[:])

    nc.compile()
    return nc


# ------------------------------------------------------------------- driver

_CACHE = {}
LAST_RES = [None, None]


def kernel(**inputs):
    return _run(inputs, trace=False)[0]


def timed_run(inputs):
    _, t1, t2 = _run(inputs, trace=True)
    return t1, t2


def _run(inputs, trace=False):
    inp = {k: np.asarray(v) for k, v in inputs.items()}
    f32 = lambda k: inp[k].astype(np.float32)
    i64 = lambda k: inp[k].astype(np.int64)

    assert np.all(inp["be"] == 0), "nonzero be not supported"

    edge_cons, edge_vars, batch_idx = i64("edge_cons"), i64("edge_vars"), i64("batch_idx")
    plan = _plan(edge_cons, edge_vars, f32("edge_attr"), batch_idx)
    CW = tuple(plan["CW"])

    key1 = ("k1", CW)
    if key1 not in _CACHE:
        _CACHE[key1] = _build_k1(list(CW))
    nc1 = _CACHE[key1]

    P_bf = (np.eye(128) - 1.0 / 128).astype(np.float32).astype(BF)
    sel24 = _sel24()
    onesm = _onesm()

    c_feat = f32("c_feat")
    v_feat = f32("v_feat")
    counts = plan["counts"]
    ntot = plan["ntot"]

    dW2, sW2 = f32("dW2"), f32("sW2")
    W2all = np.ascontiguousarray(
        np.concatenate([dW2, sW2], axis=0).reshape(
            NE + KS, 4, 128, 128).transpose(3, 0, 1, 2)).astype(BF)
    b2allT = np.ascontiguousarray(
        np.concatenate([f32("db2"), f32("sb2")], axis=0).T).astype(BF)

    in1 = []
    for c in range(NCORE):
        nos = plan["node_of_slot"][c]
        vfT = np.zeros((VF, NC_NODES), np.float32)
        real = nos >= 0
        vfT[:, real] = v_feat[nos[real]].T
        cnt = counts[c].astype(np.float32)
        padc = (PAD_G - counts[c]).astype(np.float32)
        ecidx = plan["ecidx"][c]
        used = plan["used"][c]
        cfa = np.zeros((128 * ntot, CF1), np.float32)
        cfa[used, :CF] = c_feat[ecidx[used]]
        cfa[used, CF] = 1.0
        m = dict(
            ecf=np.ascontiguousarray(
                cfa.reshape(ntot, 128, CF1).transpose(1, 0, 2).reshape(
                    128, ntot * CF1)).astype(BF),
            oea=_build_oea(plan, c),
            Wca=np.concatenate([f32("Wc"), f32("bc").reshape(1, D)],
                               axis=0).astype(BF),
            Wv=f32("Wv"), bv_col=f32("bv").reshape(D, 1),
            vfeatT=vfT,
            We_col=f32("We").reshape(D, 1),
            lng_col=f32("ln_g").reshape(D, 1), lnb_col=f32("ln_b").reshape(D, 1),
            P_bf=P_bf,
            WqT=np.ascontiguousarray(f32("Wq").T),
            tokKT=np.ascontiguousarray(f32("tokK").T),
            bq_col=f32("bq").reshape(TD, 1),
            tokV=f32("tokV").astype(BF),
            Wg_r=np.ascontiguousarray(f32("Wg").reshape(2, D, NE).transpose(1, 0, 2)),
            bg_col=f32("bg").reshape(NE, 1), eb_col=f32("ebias").reshape(NE, 1),
            alpha11=f32("alpha").reshape(1, 1).astype(BF),
            sel24=sel24, onesm=onesm,
            padc4=np.tile(padc[None, :], (128, 1)),
            invc4=np.tile((1.0 / np.maximum(cnt, 1.0))[None, :], (128, 1)),
            negpadc=(-padc).reshape(1, GPC).astype(BF),
            W2all=W2all, b2allT=b2allT,
        )
        in1.append(m)

    res1 = run_bass_kernel_spmd(nc1, in1, CORE_IDS, trace=trace)
    LAST_RES[0] = res1

    explog = np.concatenate(
        [np.asarray(res1.results[c]["explogT"]).T.astype(np.float32)
         for c in range(NCORE)], axis=0)                          # [B, NE]
    top_idx = np.argsort(-explog, axis=1, kind="stable")[:, :TOPK]  # [B, 4]
    mask = np.zeros((B, NE), np.float32)
    np.put_along_axis(mask, top_idx, 1.0, axis=1)

    if "k2" not in _CACHE:
        _CACHE["k2"] = _build_k2()
    nc2 = _CACHE["k2"]

    dW1 = f32("dW1")
    dg, dbb = f32("dg"), f32("dbb")
    sW1 = f32("sW1")
    sg, sbb = f32("sg"), f32("sbb")

    shifts_c = np.zeros((24, 2, 12), np.float32)
    for b in range(2):
        for i in range(12):
            shifts_c[12 * b + i, b, i] = 1.0
    shifts_c = shifts_c.reshape(24, 2 * 12).astype(BF)
    in2 = []
    for c in range(NCORE):
        # dedicated experts in packed (batch-major) slot order
        sel = np.array([top_idx[c * GPC + g, k] for g, k in DED_GK])  # [16]
        Esel24 = np.zeros((24, NE), np.float32)
        Gsel24 = np.zeros((GPC, 24), np.float32)
        sh05 = np.zeros((24, 1), np.float32)
        gmask24 = np.zeros((24, GPC), np.float32)
        bb24 = np.zeros((24, D), np.float32)
        wgm = np.zeros((12, NCH, 128), np.float32)
        nded = 0
        for s, (g, wi, b1i) in enumerate(SLOTS):
            gmask24[s, g] = 1.0
            if b1i >= 0:
                e = sel[nded]; nded += 1
                Esel24[s, e] = 1.0
                Gsel24[g, s] = 1.0
                bb24[s] = dbb[e]
                wgm[s % 12, s, :] = dg[e]
            else:
                j = -1 - b1i
                sh05[s, 0] = 1.0 / KS
                bb24[s] = sbb[j]
                wgm[s % 12, s, :] = sg[j]
        W1s = dW1[sel]                                  # [16, 128, 512]
        b1s = f32("db1")[sel]                           # [16, 512]
        W2Pall = np.asarray(res1.results[c]["W2Pall"]).reshape(D, NE + KS,
                                                               4, 128)
        b2Pall = np.asarray(res1.results[c]["b2Pall"]).astype(np.float32)
        slotmap = np.concatenate([sel, NE + np.arange(KS)])
        W2Psel = np.ascontiguousarray(W2Pall[:, slotmap])
        b2Psel = np.ascontiguousarray(b2Pall[:, slotmap])
        m = dict(
            vembT_bf=np.asarray(res1.results[c]["vembT"]).astype(BF),
            explog_nm=explog[c * GPC:(c + 1) * GPC],
            mask_nm=mask[c * GPC:(c + 1) * GPC],
            Esel24=Esel24, Gsel24=Gsel24, sh05=sh05,
            W1sel=np.ascontiguousarray(W1s.transpose(1, 0, 2)).astype(BF),
            sW1T=np.ascontiguousarray(sW1.transpose(1, 0, 2)).astype(BF),
            b1selT=np.ascontiguousarray(
                b1s.reshape(NSLOT, 4, 128).transpose(2, 0, 1).reshape(
                    128, NSLOT * 4)),
            sb1T=np.ascontiguousarray(
                f32("sb1").reshape(KS, 4, 128).transpose(2, 0, 1).reshape(
                    128, KS * 4)),
            W2Psel=W2Psel, b2Psel=b2Psel,
            wgm=wgm.reshape(12, NCH * 128).astype(BF),
            sel24=sel24, shifts=shifts_c,
            bb24=bb24.astype(BF),
            gmask24=gmask24,
            hW1=f32("hW1").astype(BF), hb1_col=f32("hb1").reshape(D, 1),
            hW2col=f32("hW2").reshape(D, 1).astype(BF),
            hb2=f32("hb2").reshape(1, 1),
        )
        in2.append(m)

    res2 = run_bass_kernel_spmd(nc2, in2, CORE_IDS, trace=trace)
    LAST_RES[1] = res2

    out = np.zeros(N, np.float32)
    for c in range(NCORE):
        row = np.asarray(res2.results[c]["out_row"],
                         dtype=np.float32).reshape(-1)
        nos = plan["node_of_slot"][c]
        real = nos >= 0
        out[nos[real]] = row[real]
    return out, res1.exec_time_ns, res2.exec_time_ns


# revision 48
# speedup vs baseline: 1.0065x; 1.0065x over previous
"""Trainium2 Bass kernel for nn_MoEPolicy (moe_routing).

Strategy (8 NeuronCores, SPMD, no collectives):
  - 32 graphs -> 4 graphs per core; each graph padded to 768 node slots
    (3072 padded node slots per core, 24 windows of 128).
  - Kernel 1 (per core): edge aggregation via one-hot PSUM matmuls (bf16
    one-hot scaled by edge_attr), v_emb (relu+LN), struct-token attention
    (batched, no per-node softmax max-subtract: scores are < 0.02 in
    magnitude), masked pooling, gating logits.  All heavy elementwise work
    batched into [128, 512] group ops; single activation table set
    (Ln/Exp/Relu/Square) -> one table load.
  - Host: top-4 expert selection per graph from device logits (index
    selection only), slices expert weights per core.
  - Kernel 2 (per core): route weights on device, two-pass expert
    pipeline: pass A computes all 24 expert chunk outputs (gelu on the
    scalar engine, bf16 matmuls), variances batched into one [24, 768]
    PSUM tile via selector-matmuls; one Ln+Exp gives all rstd rows; pass B
    broadcasts rstd*(route weight) via masked rank-24 matmuls and
    accumulates into the residual; task head.
All floating-point model math runs on device; the host only shards, pads,
permutes, selects indices, and casts dtypes.
"""

import sys

for _p in ("/opt/trn_rl_repo",):
    if _p not in sys.path:
        sys.path.insert(0, _p)

import numpy as np
import ml_dtypes

import concourse.bacc as bacc
import concourse.mybir as mybir
import concourse.tile as tile
from concourse.bass_utils import run_bass_kernel_spmd

F32 = mybir.dt.float32
F32R = mybir.dt.float32r
BF16 = mybir.dt.bfloat16
AF = mybir.ActivationFunctionType
ALU = mybir.AluOpType
AX = mybir.AxisListType
BF = ml_dtypes.bfloat16

# problem constants
D = 128
TD = 128
T = 64
NE = 16
KS = 2
TOPK = 4
TEMP = 0.6
B = 32
M = 10000
N = 20000
E = 160000
CF, VF, EF = 4, 6, 1

NCORE = 8
GPC = B // NCORE            # graphs per core
PAD_G = 768                 # node slots per graph
NC_NODES = GPC * PAD_G      # 3072
WPG = PAD_G // 128          # windows per graph (6)
NWIN = GPC * WPG            # 24 windows per core
NGRP = NWIN // 4            # 6 groups of 4 windows
LN_EPS = 1e-5
ISQ_TD = 1.0 / float(np.sqrt(np.float32(TD)))
CF1 = CF + 1

NSLOT = GPC * TOPK          # 16 dedicated (graph, k) slots per core
NCH = NSLOT + KS * GPC      # 24 chunk-slots (16 ded + 2 shared x 4 graphs)
HF = PAD_G // 2             # 384

CORE_IDS = list(range(NCORE))


# ---------------------------------------------------------------- host plan

def _plan(edge_cons, edge_vars, edge_attr, batch_idx):
    """Node slot assignment + edge window schedule. Pure index work."""
    order = np.argsort(batch_idx, kind="stable")
    bs = batch_idx[order]
    deg = np.bincount(edge_vars, minlength=N)

    node_of_slot = -np.ones((NCORE, NC_NODES), dtype=np.int64)
    slot_of_node = np.empty(N, dtype=np.int64)       # global slot = core*NC + s
    counts = np.zeros((NCORE, GPC), dtype=np.int64)  # real nodes per graph

    for g in range(B):
        nodes = order[np.searchsorted(bs, g, side="left"):
                      np.searchsorted(bs, g, side="right")]
        core, lg = g // GPC, g % GPC
        counts[core, lg] = len(nodes)
        if len(nodes) > PAD_G:
            raise RuntimeError(f"graph {g} has {len(nodes)} nodes > PAD_G={PAD_G}")
        # balance edge load across the graph's WPG windows
        nds = nodes[np.argsort(-deg[nodes], kind="stable")]
        wload = np.zeros(WPG, dtype=np.int64)
        wfill = np.zeros(WPG, dtype=np.int64)
        base = lg * PAD_G
        for nd in nds:
            cand = np.where(wfill < 128)[0]
            w = cand[np.argmin(wload[cand])]
            s = base + w * 128 + wfill[w]
            node_of_slot[core, s] = nd
            slot_of_node[nd] = core * NC_NODES + s
            wload[w] += deg[nd]
            wfill[w] += 1

    # edges -> (core, window, lane j)
    eslot = slot_of_node[edge_vars]
    ecore = eslot // NC_NODES
    es = eslot % NC_NODES
    ewin = es // 128
    ej = es % 128

    # tiles per window position, shared across cores
    cw = np.zeros((NCORE, NWIN), dtype=np.int64)
    per = {}
    for c in range(NCORE):
        sel = np.where(ecore == c)[0]
        for w in range(NWIN):
            ews = sel[ewin[sel] == w]
            per[(c, w)] = ews
            cw[c, w] = max(1, -(-len(ews) // 128))
    CW = cw.max(axis=0)
    ntot = int(CW.sum())

    ecidx = np.zeros((NCORE, 128 * ntot), dtype=np.int64)   # cons index per slot
    used = np.zeros((NCORE, 128 * ntot), dtype=bool)
    vloc = np.full((NCORE, 128 * ntot), -1.0, dtype=np.float32)
    eav = np.zeros((NCORE, 128 * ntot), dtype=np.float32)
    offs = np.concatenate([[0], np.cumsum(CW)]) * 128
    ea_flat = edge_attr.reshape(-1).astype(np.float32)
    for c in range(NCORE):
        for w in range(NWIN):
            ews = per[(c, w)]
            o = offs[w]
            ecidx[c, o:o + len(ews)] = edge_cons[ews]
            used[c, o:o + len(ews)] = True
            vloc[c, o:o + len(ews)] = ej[ews]
            eav[c, o:o + len(ews)] = ea_flat[ews]

    return dict(node_of_slot=node_of_slot, counts=counts, CW=CW.tolist(),
                ntot=ntot, ecidx=ecidx, used=used, vloc=vloc, eav=eav)


def _build_oea(plan, c):
    """One-hot (scaled by edge_attr) [128 lanes, tile, 128 nodes], bf16."""
    ntot = plan["ntot"]
    vloc = plan["vloc"][c].reshape(ntot, 128)
    eav = plan["eav"][c].reshape(ntot, 128)
    arr = np.zeros((128, ntot, 128), np.float32)   # [lane, tile, n]
    t_i, p_i = np.nonzero(vloc >= 0)
    arr[p_i, t_i, vloc[t_i, p_i].astype(np.int64)] = eav[t_i, p_i]
    return np.ascontiguousarray(arr.reshape(128, ntot * 128)).astype(BF)


def _sel24():
    """[128, 24, 24] bf16: SEL24[:, w, j] = (j == w)."""
    s = np.zeros((128, 24, 24), np.float32)
    for w in range(24):
        s[:, w, w] = 1.0
    return s.reshape(128, 24 * 24).astype(BF)


def _onesm():
    """[24, 24, 128] bf16: ONESM[r, w, :] = (r == w)."""
    s = np.zeros((24, 24, 128), np.float32)
    for w in range(24):
        s[w, w, :] = 1.0
    return s.reshape(24, 24 * 128).astype(BF)




# two batches: batch b covers graphs {2b, 2b+1}; 8 dedicated + 4 shared each.
# slot s order: [b0: ded g0k0..g1k3, sh j0g0, j0g1, j1g0, j1g1] then batch 1.
def _slots():
    out = []   # per slot: (graph, wi, b1idx)  wi: index into W2Psel/b2Psel
    nded = 0
    for b in range(2):
        for g in (2 * b, 2 * b + 1):
            for k in range(TOPK):
                out.append((g, nded, nded))
                nded += 1
        for j in range(KS):
            for g in (2 * b, 2 * b + 1):
                out.append((g, NSLOT + j, -1 - j))
    return out


SLOTS = _slots()
DED_GK = []   # (graph, k) in packed ded order
for b in range(2):
    for g in (2 * b, 2 * b + 1):
        for k in range(TOPK):
            DED_GK.append((g, k))

# ------------------------------------------------------------- build kernel1

DEBUG_K1 = False


def _build_k1(CW):
    ntot = int(sum(CW))
    nc = bacc.Bacc("TRN2", target_bir_lowering=False, debug=False,
                   num_devices=NCORE)

    def din(name, shape, dt=F32):
        return nc.dram_tensor(name, shape, dt, kind="ExternalInput")

    ecf_i = din("ecf", [128, ntot * CF1], BF16)
    oea_i = din("oea", [128, ntot * 128], BF16)
    Wca_i = din("Wca", [CF1, D], BF16)
    Wv_i = din("Wv", [VF, D])
    bv_i = din("bv_col", [D, 1])
    vfT_i = din("vfeatT", [VF, NC_NODES])
    We_i = din("We_col", [D, 1])
    lng_i = din("lng_col", [D, 1])
    lnb_i = din("lnb_col", [D, 1])
    P_i = din("P_bf", [128, 128], BF16)
    WqT_i = din("WqT", [TD, D])
    tokKT_i = din("tokKT", [TD, T])
    bq_i = din("bq_col", [TD, 1])
    tokV_i = din("tokV", [T, TD], BF16)
    Wg_i = din("Wg_r", [D, 2, NE])
    bg_i = din("bg_col", [NE, 1])
    eb_i = din("eb_col", [NE, 1])
    al_i = din("alpha11", [1, 1], BF16)
    sel24_i = din("sel24", [128, 24 * 24], BF16)
    onesm_i = din("onesm", [24, 24 * 128], BF16)
    padc4_i = din("padc4", [128, GPC])
    invc4_i = din("invc4", [128, GPC])
    negpadc_i = din("negpadc", [1, GPC], BF16)
    W2a_i = din("W2all", [D, NSLOT + KS, 4, 128], BF16)
    b2a_i = din("b2allT", [D, NSLOT + KS], BF16)

    vembT_o = nc.dram_tensor("vembT", [D, NC_NODES], BF16, kind="ExternalOutput")
    exlg_o = nc.dram_tensor("explogT", [NE, GPC], F32, kind="ExternalOutput")
    W2P_o = nc.dram_tensor("W2Pall", [D, (NSLOT + KS) * 4 * 128], BF16,
                           kind="ExternalOutput")
    b2P_o = nc.dram_tensor("b2Pall", [D, NSLOT + KS], F32,
                           kind="ExternalOutput")

    offs = np.concatenate([[0], np.cumsum(CW)]).astype(int)
    goffs = [int(offs[4 * g]) for g in range(NGRP + 1)]   # tile offsets per group

    with tile.TileContext(nc) as tc:
        with (
            tc.tile_pool(name="cp", bufs=1) as cp,
            tc.tile_pool(name="oh", bufs=2) as ohp,
            tc.tile_pool(name="wk", bufs=3) as wk,
            tc.tile_pool(name="sm", bufs=4) as smp,
            tc.tile_pool(name="ps", bufs=1, space="PSUM") as ps,
        ):
            PS_BUFS = {"g1": 2, "mm": 3, "pa": 2}
            _ld = [0]
            def load(ap_dram, shape, dt=F32):
                _ld[0] += 1
                t_ = cp.tile(shape, dt, tag=f"cst{_ld[0]}", name=f"cst{_ld[0]}")
                src_ap = ap_dram[:]
                if dt == F32R:
                    src_ap = src_ap.bitcast(F32R)
                nc.sync.dma_start(t_[:], src_ap)
                return t_

            ecf_s = load(ecf_i, [128, ntot * CF1], BF16)
            # group 0/1 one-hot DMAs first: they head the critical path
            oea_pre = []
            for _g in range(2):
                gt0, gt1 = goffs[_g], goffs[_g + 1]
                _t = ohp.tile([128, 32 * 128], BF16, tag="oea", name="oeaw")
                nc.sync.dma_start(_t[:, :(gt1 - gt0) * 128],
                                  oea_i[:, gt0 * 128:gt1 * 128])
                oea_pre.append(_t)
            Wca_s = load(Wca_i, [CF1, D], BF16)
            Wv_s = load(Wv_i, [VF, D], F32R)
            bv_s = load(bv_i, [D, 1])
            vfT_s = load(vfT_i, [VF, NC_NODES], F32R)
            We_s = load(We_i, [D, 1])
            lng_s = load(lng_i, [D, 1])
            lnb_s = load(lnb_i, [D, 1])
            P_s = load(P_i, [128, 128], BF16)
            WqT_s = load(WqT_i, [TD, D], F32R)
            tKT_s = load(tokKT_i, [TD, T], F32R)
            bq_s = load(bq_i, [TD, 1], F32R)
            tV_s = load(tokV_i, [T, TD], BF16)
            Wg_s = load(Wg_i, [D, 2, NE], F32R)
            bg_s = load(bg_i, [NE, 1])
            eb_s = load(eb_i, [NE, 1])
            al_s = load(al_i, [1, 1], BF16)
            sel24 = load(sel24_i, [128, 24, 24], BF16)
            onesm = load(onesm_i, [24, 24, 128], BF16)
            padc4 = load(padc4_i, [128, GPC])
            invc4 = load(invc4_i, [128, GPC])
            negpadc = load(negpadc_i, [1, GPC], BF16)

            onesr_bf = cp.tile([1, 128], BF16, name="onesr_bf")
            nc.vector.memset(onesr_bf[:], 1.0)
            onesc_bf = cp.tile([128, 1], BF16, name="onesc_bf")
            nc.vector.memset(onesc_bf[:], 1.0)
            eps24 = cp.tile([24, 1], F32, name="eps24")
            nc.vector.memset(eps24[:], LN_EPS)

            # persistent big tiles
            c_all = cp.tile([128, NGRP, 4, 128], F32, name="c_all")
            v0b_all = cp.tile([128, NGRP, 512], F32, name="v0b_all")
            vembT_s = cp.tile([128, NWIN, 128], BF16, name="vembT_s")
            wsum = cp.tile([128, NWIN], F32, name="wsum")
            varsb = cp.tile([24, NGRP, 128], F32, name="varsb")
            rstd24 = cp.tile([24, NGRP, 128], BF16, name="rstd24")
            Wp_s = cp.tile([D, T], BF16, name="Wp_s")       # Wq @ tokK^T
            bqK_s = cp.tile([1, T], BF16, name="bqK_s")

            # ---- prologue: W' = Wq @ tokK^T  [D, T]; bqK = bq^T tokK^T
            pWp = ps.tile([128, 512], F32, tag="mm", name="pWp",
                          bufs=PS_BUFS["mm"])
            nc.tensor.matmul(pWp[:, :T], WqT_s[:], tKT_s[:], start=True, stop=True)
            nc.vector.tensor_copy(Wp_s[:], pWp[:, :T])
            pbq = ps.tile([NE, 512], F32, tag="g1", name="pbq",
                          bufs=PS_BUFS["g1"])
            nc.tensor.matmul(pbq[:1, :T], bq_s[:], tKT_s[:], start=True, stop=True)
            nc.vector.tensor_copy(bqK_s[:], pbq[:1, :T])

            # ---- v0 for all groups up front (independent of edges)
            for grp in range(NGRP):
                pv0 = ps.tile([128, 512], F32, tag="mm", name="pv0",
                              bufs=PS_BUFS["mm"])
                nc.tensor.matmul(pv0[:], Wv_s[:],
                                 vfT_s[:, grp * 512:(grp + 1) * 512],
                                 start=True, stop=True)
                nc.vector.tensor_scalar(v0b_all[:, grp, :], pv0[:], bv_s[:],
                                        None, ALU.add)

            # ---- pad-column head: x=relu(bv); c=P x; var -> varsb[0, 5, 0]
            z0 = smp.tile([128, 1], F32, tag="pad", name="z0")
            nc.vector.memset(z0[:], 0.0)
            xp = smp.tile([128, 1], BF16, tag="padb", name="xp")
            nc.scalar.activation(xp[:], z0[:], AF.Relu, bias=bv_s[:])
            pcp = ps.tile([128, 512], F32, tag="mm", name="pcp",
                          bufs=PS_BUFS["mm"])
            nc.tensor.matmul(pcp[:, :1], P_s[:], xp[:], start=True, stop=True)
            cgp = smp.tile([128, 1], F32, tag="pad", name="cgp")
            nc.vector.tensor_scalar(cgp[:], pcp[:, :1], lng_s[:], None, ALU.mult)
            sqp = smp.tile([128, 1], BF16, tag="padb", name="sqp")
            nc.vector.tensor_tensor(sqp[:], cgp[:], cgp[:], ALU.mult)
            pvp = ps.tile([NE, 512], F32, tag="g1", name="pvp",
                          bufs=PS_BUFS["g1"])
            nc.tensor.matmul(pvp[:1, :1], onesc_bf[:], sqp[:], start=True, stop=True)
            nc.vector.tensor_copy(varsb[0:1, NGRP - 1:NGRP, 0:1], pvp[:1, :1])

            # ---- phase 1, software pipelined: G1(g) | midA(g-1) | midB(g-2)
            def midA(grp):
                pT1 = ps.tile([128, 512], F32, tag="mm", name="pT1",
                              bufs=PS_BUFS["mm"])
                nc.tensor.matmul(pT1[:], Wca_s[:], G1t[grp][:],
                                 start=True, stop=True)
                s_sb = wk.tile([128, 512], F32, tag="s", name="s_sb")
                nc.vector.scalar_tensor_tensor(
                    s_sb[:], pT1[:], We_s[:], v0b_all[:, grp, :],
                    ALU.mult, ALU.add)
                x_bf = wk.tile([128, 512], BF16, tag="x", name="x_bf")
                nc.scalar.activation(x_bf[:], s_sb[:], AF.Relu)
                pc = ps.tile([128, 512], F32, tag="mm", name="pc",
                             bufs=PS_BUFS["mm"])
                nc.tensor.matmul(pc[:], P_s[:], x_bf[:], start=True, stop=True)
                nc.vector.tensor_scalar(
                    c_all[:, grp, :, :], pc[:], lng_s[:], None, ALU.mult)
                sqt = wk.tile([128, 4, 128], BF16, tag="sq", name="sqt")
                nc.vector.tensor_tensor(sqt[:], c_all[:, grp, :, :],
                                        c_all[:, grp, :, :], ALU.mult)
                sq_t[grp] = sqt

            def midB(grp):
                pvarg = ps.tile([24, 128], F32, tag="g1", name="pvarg",
                                bufs=PS_BUFS["g1"])
                for wi in range(4):
                    w = grp * 4 + wi
                    nc.tensor.matmul(pvarg[:], sel24[:, w, :],
                                     sq_t[grp][:, wi, :],
                                     start=(wi == 0), stop=(wi == 3))
                nc.vector.tensor_copy(varsb[:, grp, :], pvarg[:])

            G1t = [None] * NGRP
            sq_t = [None] * NGRP
            for grp in range(NGRP):
                gt0, gt1 = goffs[grp], goffs[grp + 1]
                nt = gt1 - gt0
                if grp < 2:
                    oeaw = oea_pre[grp]
                else:
                    oeaw = ohp.tile([128, 32 * 128], BF16, tag="oea",
                                    name="oeaw")
                    nc.sync.dma_start(oeaw[:, :nt * 128],
                                      oea_i[:, gt0 * 128:gt1 * 128])
                pG1 = ps.tile([5, 512], F32, tag="g1", name="pG1",
                              bufs=PS_BUFS["g1"])
                for wi in range(4):
                    w = grp * 4 + wi
                    for t_ in range(int(CW[w])):
                        gt = int(offs[w]) + t_
                        lt = gt - gt0
                        nc.tensor.matmul(
                            pG1[:CF1, wi * 128:(wi + 1) * 128],
                            ecf_s[:, gt * CF1:(gt + 1) * CF1],
                            oeaw[:, lt * 128:(lt + 1) * 128],
                            start=(t_ == 0), stop=(t_ == int(CW[w]) - 1))
                G1sb = wk.tile([CF1, 512], BF16, tag="g1sb", bufs=2, name="G1sb")
                nc.vector.tensor_copy(G1sb[:], pG1[:CF1, :])
                G1t[grp] = G1sb
                if grp >= 1:
                    midA(grp - 1)
                if grp >= 2:
                    midB(grp - 2)
            midA(NGRP - 1)
            midB(NGRP - 2)
            midB(NGRP - 1)

            # W2 fold inputs: issue DMA now so it rides behind the oea loads
            W2a_s = cp.tile([D, NSLOT + KS, 4, 128], BF16, name="W2a_s")
            nc.sync.dma_start(W2a_s[:], W2a_i[:])
            b2a_s = cp.tile([D, NSLOT + KS], BF16, name="b2a_s")
            nc.sync.dma_start(b2a_s[:], b2a_i[:])

            # ---- rstd for all windows (incl pad at [0, NGRP-1, 0])
            lnv = wk.tile([24, NGRP, 128], F32, tag="lnv", bufs=1, name="lnv")
            nc.scalar.activation(lnv[:], varsb[:], AF.Ln,
                                 bias=eps24[:], scale=1.0 / D)
            nc.scalar.activation(rstd24[:], lnv[:], AF.Exp, scale=-0.5)

            # ---- pad-column tail (uses batched pad rstd)
            pbb = ps.tile([128, 512], F32, tag="mm", name="pbb",
                          bufs=PS_BUFS["mm"])
            nc.tensor.matmul(pbb[:, :1], onesr_bf[:],
                             rstd24[0:1, NGRP - 1, 0:1], start=True, stop=True)
            up = smp.tile([128, 1], F32, tag="pad", name="up")
            nc.vector.tensor_tensor(up[:], cgp[:], pbb[:, :1], ALU.mult)
            vp = smp.tile([128, 1], BF16, tag="padb", name="vp")
            nc.vector.tensor_scalar(vp[:], up[:], lnb_s[:], None, ALU.add)
            pscp = ps.tile([NE, 512], F32, tag="g1", name="pscp",
                           bufs=PS_BUFS["g1"])
            nc.tensor.matmul(pscp[:1, :T], vp[:], Wp_s[:], start=True, stop=False)
            nc.tensor.matmul(pscp[:1, :T], onesr_bf[:, :1], bqK_s[:],
                             start=False, stop=True)
            exps = smp.tile([1, T], F32, tag="padr", name="exps")
            nc.scalar.activation(exps[:], pscp[:1, :T], AF.Exp, scale=ISQ_TD)
            smsum = smp.tile([1, 1], F32, tag="pads", name="smsum")
            nc.vector.tensor_reduce(smsum[:], exps[:], AX.X, ALU.add)
            rcp = smp.tile([1, 1], F32, tag="pads", name="rcp")
            nc.vector.reciprocal(rcp[:], smsum[:])
            wtsp = smp.tile([1, T], BF16, tag="padr", name="wtsp")
            nc.vector.tensor_scalar(wtsp[:], exps[:], rcp[:], None, ALU.mult)

            # ---- phase 2 + struct scores, software pipelined per group
            R = ps.tile([64, 8], F32, tag="g1", name="R", bufs=PS_BUFS["g1"])

            def rowsums(grp):
                for wi in range(4):
                    w = grp * 4 + wi
                    g, j = w // WPG, w % WPG
                    nc.tensor.matmul(R[:T, g:g + 1], wts_t[grp][:, wi, :],
                                     onesc_bf[:], start=(j == 0),
                                     stop=(j == WPG - 1))

            wts_t = [None] * NGRP
            for grp in range(NGRP):
                pA = ps.tile([128, 4, 128], F32, tag="pa", name="pA",
                             bufs=PS_BUFS["pa"])
                for wi in range(4):
                    w = grp * 4 + wi
                    nc.tensor.matmul(pA[:, wi, :], onesm[:, w, :],
                                     rstd24[:, grp, :], start=True, stop=True)
                u_sb = wk.tile([128, 4, 128], F32, tag="u", name="u_sb")
                nc.vector.tensor_tensor(u_sb[:], c_all[:, grp, :, :], pA[:],
                                        ALU.mult)
                nc.scalar.activation(vembT_s[:, 4 * grp:4 * grp + 4, :],
                                      u_sb[:], AF.Identity, bias=lnb_s[:])
                nc.vector.tensor_reduce(wsum[:, 4 * grp:4 * grp + 4],
                                        u_sb[:], AX.X, ALU.add)
                psc = ps.tile([128, 4, 64], F32, tag="pa", name="psc",
                              bufs=PS_BUFS["pa"])
                for wi in range(4):
                    w = grp * 4 + wi
                    nc.tensor.matmul(psc[:, wi, :], vembT_s[:, w, :], Wp_s[:],
                                     start=True, stop=False)
                    nc.tensor.matmul(psc[:, wi, :], onesr_bf[:], bqK_s[:],
                                     start=False, stop=True)
                ex = wk.tile([128, 4, 64], BF16, tag="ex", bufs=2, name="ex")
                nc.scalar.activation(ex[:], psc[:], AF.Exp, scale=ISQ_TD)
                sme = smp.tile([128, 4], F32, tag="sme", bufs=3, name="sme")
                nc.vector.tensor_reduce(sme[:], ex[:], AX.X, ALU.add)
                rce = smp.tile([128, 4], F32, tag="rce", bufs=3, name="rce")
                nc.vector.reciprocal(rce[:], sme[:])
                wts = wk.tile([128, 4, 64], BF16, tag="wts", bufs=3, name="wts")
                for wi in range(4):
                    nc.vector.tensor_scalar(wts[:, wi, :], ex[:, wi, :],
                                            rce[:, wi:wi + 1], None, ALU.mult)
                wts_t[grp] = wts
                if grp >= 1:
                    rowsums(grp - 1)
            rowsums(NGRP - 1)
            nc.tensor.matmul(R[:T, GPC:2 * GPC], wtsp[:], negpadc[:],
                             start=True, stop=True)

            nc.sync.dma_start(vembT_o[:], vembT_s[:])

            # ---- struct pooling
            Rsb = smp.tile([64, 2 * GPC], F32, tag="Rsb", bufs=1, name="Rsb")
            nc.vector.tensor_copy(Rsb[:], R[:T, :2 * GPC])
            Rc = smp.tile([64, GPC], BF16, tag="Rc", bufs=1, name="Rc")
            nc.vector.tensor_tensor(Rc[:], Rsb[:, :GPC], Rsb[:, GPC:2 * GPC],
                                    ALU.add)
            pstr = ps.tile([128, 512], F32, tag="mm", name="pstr",
                           bufs=PS_BUFS["mm"])
            nc.tensor.matmul(pstr[:, :GPC], tV_s[:], Rc[:], start=True, stop=True)
            strT = smp.tile([128, GPC], F32R, tag="strT", bufs=1, name="strT")
            with nc.allow_low_precision(reason="gating rhs f32r"):
                nc.vector.tensor_tensor(strT[:], pstr[:, :GPC], invc4[:],
                                        ALU.mult)

            # ---- graph embedding pooling with pad correction
            gsum = smp.tile([128, GPC], F32, tag="gsum", bufs=1, name="gsum")
            for g in range(GPC):
                nc.vector.tensor_reduce(gsum[:, g:g + 1],
                                        wsum[:, g * WPG:(g + 1) * WPG],
                                        AX.X, ALU.add)
            t3 = smp.tile([128, GPC], F32, tag="t3", bufs=1, name="t3")
            nc.vector.tensor_scalar(t3[:], padc4[:], up[:], None, ALU.mult)
            t4 = smp.tile([128, GPC], F32, tag="t4", bufs=1, name="t4")
            nc.vector.tensor_tensor(t4[:], gsum[:], t3[:], ALU.subtract)
            t5 = smp.tile([128, GPC], F32, tag="t5", bufs=1, name="t5")
            nc.vector.tensor_tensor(t5[:], t4[:], invc4[:], ALU.mult)
            gembT = smp.tile([128, GPC], F32R, tag="gembT", bufs=1, name="gembT")
            with nc.allow_low_precision(reason="gating rhs f32r"):
                nc.vector.tensor_scalar(gembT[:], t5[:], lnb_s[:], None, ALU.add)

            # ---- gating logits -> exp(logits)
            pl = ps.tile([NE, 512], F32, tag="g1", name="pl", bufs=PS_BUFS["g1"])
            nc.tensor.matmul(pl[:, :GPC], Wg_s[:, 0, :], gembT[:],
                             start=True, stop=False)
            nc.tensor.matmul(pl[:, :GPC], Wg_s[:, 1, :], strT[:],
                             start=False, stop=True)
            pa_ = ps.tile([128, 512], F32, tag="mm", name="pa_",
                          bufs=PS_BUFS["mm"])
            nc.tensor.matmul(pa_[:NE, :1], onesr_bf[:, :NE], al_s[:],
                             start=True, stop=True)
            acol = smp.tile([NE, 1], F32, tag="acol", bufs=1, name="acol")
            nc.vector.tensor_copy(acol[:], pa_[:NE, :1])
            lg1 = smp.tile([NE, GPC], F32, tag="lg1", bufs=1, name="lg1")
            nc.vector.tensor_scalar(lg1[:], pl[:, :GPC], bg_s[:], None, ALU.add)
            lg2 = smp.tile([NE, GPC], F32, tag="lg2", bufs=1, name="lg2")
            nc.vector.tensor_scalar(lg2[:], lg1[:], acol[:], 1.0 / TEMP,
                                    ALU.mult, ALU.mult)
            lg3 = smp.tile([NE, GPC], F32, tag="lg3", bufs=1, name="lg3")
            nc.vector.tensor_scalar(lg3[:], lg2[:], eb_s[:], None, ALU.add)
            exlg = smp.tile([NE, GPC], F32, tag="exlg", bufs=1, name="exlg")
            nc.scalar.activation(exlg[:], lg3[:], AF.Exp)
            nc.sync.dma_start(exlg_o[:], exlg[:])

            # ---- W2 fold for all experts: W2P = (W2_chunk @ P), h-major
            W2P = cp.tile([128, NSLOT + KS, 4, 128], BF16, name="W2P")
            for s in range(NSLOT + KS):
                pw = ps.tile([128, 512], F32, tag="mm", name="pw",
                             bufs=PS_BUFS["mm"])
                for c in range(4):
                    nc.tensor.matmul(pw[:, c * 128:(c + 1) * 128],
                                     W2a_s[:, s, c, :], P_s[:],
                                     start=True, stop=True)
                nc.scalar.copy(W2P[:, s, :, :], pw[:])
                nc.sync.dma_start(W2P_o[:, s * 512:(s + 1) * 512],
                                  W2P[:, s, :, :])
            pb2 = ps.tile([128, 512], F32, tag="mm", name="pb2",
                          bufs=PS_BUFS["mm"])
            nc.tensor.matmul(pb2[:, :NSLOT + KS], P_s[:], b2a_s[:],
                             start=True, stop=True)
            b2P = cp.tile([D, NSLOT + KS], F32, name="b2P")
            nc.vector.tensor_copy(b2P[:], pb2[:, :NSLOT + KS])
            nc.sync.dma_start(b2P_o[:], b2P[:])

    nc.compile()
    return nc


# ------------------------------------------------------------- build kernel2

def _build_k2():
    nc = bacc.Bacc("TRN2", target_bir_lowering=False, debug=False,
                   num_devices=NCORE)

    def din(name, shape, dt=F32):
        return nc.dram_tensor(name, shape, dt, kind="ExternalInput")

    vembT_i = din("vembT_bf", [D, NC_NODES], BF16)
    explog_i = din("explog_nm", [GPC, NE])
    mask_i = din("mask_nm", [GPC, NE])
    Esel_i = din("Esel24", [24, NE])
    Gsel_i = din("Gsel24", [GPC, 24])
    sh05_i = din("sh05", [24, 1])
    W1sel_i = din("W1sel", [D, NSLOT, 4 * D], BF16)
    sW1_i = din("sW1T", [D, KS, 4 * D], BF16)
    b1selT_i = din("b1selT", [128, NSLOT * 4])
    sb1T_i = din("sb1T", [128, KS * 4])
    W2P_i = din("W2Psel", [D, NSLOT + KS, 4, 128], BF16)
    b2P_i = din("b2Psel", [D, NSLOT + KS])
    wgm_i = din("wgm", [12, NCH * 128], BF16)
    sel24_i = din("sel24", [128, 24 * 24], BF16)
    shifts_i = din("shifts", [24, 2 * 12], BF16)
    bb24_i = din("bb24", [24, D], BF16)
    gmask_i = din("gmask24", [24, GPC])
    hW1_i = din("hW1", [D, D], BF16)
    hb1_i = din("hb1_col", [D, 1])
    hW2_i = din("hW2col", [D, 1], BF16)
    hb2_i = din("hb2", [1, 1])

    out_o = nc.dram_tensor("out_row", [1, NC_NODES], F32, kind="ExternalOutput")

    with tile.TileContext(nc) as tc:
        with (
            tc.tile_pool(name="cp", bufs=1) as cp,
            tc.tile_pool(name="wk", bufs=3) as wk,
            tc.tile_pool(name="sm", bufs=4) as smp,
            tc.tile_pool(name="ps", bufs=1, space="PSUM") as ps,
        ):
            PS_BUFS = {"ph": 3, "pc": 3, "var": 1}
            _ld = [0]
            def load(ap_dram, shape, dt=F32):
                _ld[0] += 1
                t_ = cp.tile(shape, dt, tag=f"cst{_ld[0]}", name=f"cst{_ld[0]}")
                src_ap = ap_dram[:]
                if dt == F32R:
                    src_ap = src_ap.bitcast(F32R)
                nc.sync.dma_start(t_[:], src_ap)
                return t_

            # batch-0 slot data first in the DMA queue
            vembT = cp.tile([D, NC_NODES], BF16, tag="cvembT", name="vembT")
            nc.sync.dma_start(vembT[:, :NC_NODES // 2],
                              vembT_i[:, :NC_NODES // 2])
            W1 = cp.tile([D, NSLOT, 4 * D], BF16, tag="cW1", name="W1")
            nc.sync.dma_start(W1[:, :8, :], W1sel_i[:, :8, :])
            b1T = load(b1selT_i, [128, NSLOT * 4])
            sb1T = load(sb1T_i, [128, KS * 4])
            b2P = load(b2P_i, [D, NSLOT + KS])
            sW1 = load(sW1_i, [D, KS, 4 * D], BF16)
            W2P = cp.tile([D, NSLOT + KS, 4, 128], BF16, tag="cW2P",
                          name="W2P")
            nc.sync.dma_start(W2P[:, :8, :, :], W2P_i[:, :8, :, :])
            nc.sync.dma_start(W2P[:, NSLOT:, :, :], W2P_i[:, NSLOT:, :, :])
            wgm = load(wgm_i, [12, NCH, 128], BF16)
            shifts = load(shifts_i, [24, 2, 12], BF16)
            sel24 = load(sel24_i, [128, 24, 24], BF16)
            exlg = load(explog_i, [GPC, NE])
            msk = load(mask_i, [GPC, NE])
            Esel = load(Esel_i, [24, NE])
            Gsel = load(Gsel_i, [GPC, 24], F32R)
            sh05 = load(sh05_i, [24, 1])
            bb24 = load(bb24_i, [24, D], BF16)
            gmask = load(gmask_i, [24, GPC])
            hW1 = load(hW1_i, [D, D], BF16)
            hb1 = load(hb1_i, [D, 1])
            hW2 = load(hW2_i, [D, 1], BF16)
            hb2 = load(hb2_i, [1, 1])
            # batch-1 slot data at the tail of the DMA queue
            nc.sync.dma_start(vembT[:, NC_NODES // 2:],
                              vembT_i[:, NC_NODES // 2:])
            nc.sync.dma_start(W1[:, 8:, :], W1sel_i[:, 8:, :])
            nc.sync.dma_start(W2P[:, 8:NSLOT, :, :], W2P_i[:, 8:NSLOT, :, :])

            eps24 = cp.tile([24, 1], F32, name="eps24")
            nc.vector.memset(eps24[:], LN_EPS)

            acc = cp.tile([D, NC_NODES], F32, name="acc")
            cbS = cp.tile([128, NCH, 2, HF], BF16, name="cbS")
            out_sb = cp.tile([1, NC_NODES], F32, name="out_sb")

            # ---- route weights on device (exp(logits) comes from k1)
            sme = smp.tile([GPC, 1], F32, tag="sme", bufs=1, name="sme")
            nc.vector.tensor_reduce(sme[:], exlg[:], AX.X, ALU.add)
            rce = smp.tile([GPC, 1], F32, tag="rce", bufs=1, name="rce")
            nc.vector.reciprocal(rce[:], sme[:])
            w_sm = smp.tile([GPC, NE], F32, tag="w_sm", bufs=1, name="w_sm")
            nc.vector.tensor_scalar(w_sm[:], exlg[:], rce[:], None, ALU.mult)
            wm = smp.tile([GPC, NE], F32, tag="wm", bufs=1, name="wm")
            nc.vector.tensor_tensor(wm[:], w_sm[:], msk[:], ALU.mult)
            s2_ = smp.tile([GPC, 1], F32, tag="s2_", bufs=1, name="s2_")
            nc.vector.tensor_reduce(s2_[:], wm[:], AX.X, ALU.add)
            s2e = smp.tile([GPC, 1], F32, tag="s2e", bufs=1, name="s2e")
            nc.gpsimd.tensor_scalar(s2e[:], s2_[:], 1e-12, None, ALU.add)
            rc2 = smp.tile([GPC, 1], F32, tag="rc2", bufs=1, name="rc2")
            nc.vector.reciprocal(rc2[:], s2e[:])
            route = smp.tile([GPC, NE], F32, tag="route", bufs=1, name="route")
            nc.vector.tensor_scalar(route[:], wm[:], rc2[:], None, ALU.mult)
            route_r = smp.tile([GPC, NE], F32R, tag="route_r", bufs=1,
                               name="route_r")
            with nc.allow_low_precision(reason="route f32r view"):
                nc.vector.tensor_copy(route_r[:], route[:])

            pR2 = ps.tile([128, 512], F32, tag="pc", name="pR2",
                          bufs=PS_BUFS["pc"])
            nc.tensor.matmul(pR2[:24, :NE], Gsel[:], route_r[:],
                             start=True, stop=True)
            r2e = smp.tile([24, NE], F32, tag="r2e", bufs=1, name="r2e")
            nc.vector.tensor_tensor(r2e[:], pR2[:24, :NE], Esel[:], ALU.mult)
            wc24 = smp.tile([24, 1], F32, tag="wc24", bufs=1, name="wc24")
            nc.vector.tensor_reduce(wc24[:], r2e[:], AX.X, ALU.add)
            wcol24 = cp.tile([24, 1], F32, name="wcol24")
            nc.vector.tensor_tensor(wcol24[:], wc24[:], sh05[:], ALU.add)
            wcol24_bf = cp.tile([24, 1], BF16, name="wcol24_bf")
            nc.vector.tensor_copy(wcol24_bf[:], wcol24[:])
            wcolb = []
            for b in range(2):
                pwc = ps.tile([128, 512], F32, tag="pc", name="pwc",
                              bufs=PS_BUFS["pc"])
                nc.tensor.matmul(pwc[:12, :1], shifts[:, b, :], wcol24_bf[:],
                                 start=True, stop=True)
                wcb = cp.tile([12, 1], F32, name=f"wcb{b}")
                nc.vector.tensor_copy(wcb[:], pwc[:12, :1])
                wcolb.append(wcb)

            # per-graph LN bias columns: biasg = bb24^T @ (gmask * wcol24)
            wsel24 = smp.tile([24, GPC], BF16, tag="wsel", bufs=1,
                              name="wsel24")
            nc.vector.tensor_scalar(wsel24[:], gmask[:], wcol24[:], None,
                                    ALU.mult)
            pbg = ps.tile([128, 512], F32, tag="pc", name="pbg",
                          bufs=PS_BUFS["pc"])
            nc.tensor.matmul(pbg[:, :GPC], bb24[:], wsel24[:],
                             start=True, stop=True)
            biasg = cp.tile([D, GPC], F32, name="biasg")
            nc.vector.tensor_copy(biasg[:], pbg[:, :GPC])

            # ---- expert pipeline, two batches of 12 slots; pass B / head of
            # batch b overlaps pass A of batch b+1
            pvar = ps.tile([12, 2, 512], F32, tag="var", name="pvar",
                           bufs=PS_BUFS["var"])
            sq_t = [None] * NCH
            rstdw_t = [None, None]
            first = set()

            def emit_front(s, local, last_local):
                g, wi, b1i = SLOTS[s]
                off = g * PAD_G
                if b1i >= 0:
                    W1t = W1[:, b1i, :]
                    b1c = b1T[:, b1i * 4:(b1i + 1) * 4]
                else:
                    j = -1 - b1i
                    W1t = sW1[:, j, :]
                    b1c = sb1T[:, j * 4:(j + 1) * 4]
                hTns = []
                for h in range(2):
                    for c in range(4):
                        ph = ps.tile([128, HF], F32, tag="ph", name="ph",
                                     bufs=PS_BUFS["ph"])
                        nc.tensor.matmul(
                            ph[:], W1t[:, c * 128:(c + 1) * 128],
                            vembT[:, off + h * HF:off + (h + 1) * HF],
                            start=True, stop=True)
                        hTn = wk.tile([128, HF], BF16, tag="hTn", bufs=10,
                                      name="hTn")
                        nc.scalar.activation(hTn[:], ph[:], AF.Gelu,
                                             bias=b1c[:, c:c + 1])
                        hTns.append(hTn)
                if local >= 1:
                    emit_var(s - 1, local - 1, last_local)
                for h in range(2):
                    pc_ = ps.tile([128, HF], F32, tag="pc", name="pc_",
                                  bufs=PS_BUFS["pc"])
                    for c in range(4):
                        nc.tensor.matmul(pc_[:], W2P[:, wi, c, :],
                                         hTns[h * 4 + c][:],
                                         start=(c == 0), stop=(c == 3))
                    nc.vector.tensor_scalar(cbS[:, s, h, :], pc_[:],
                                            b2P[:, wi:wi + 1], None, ALU.add)
                sqt = wk.tile([128, 2, HF], BF16, tag="sq", bufs=3, name="sqt")
                nc.vector.tensor_tensor(sqt[:], cbS[:, s, :, :],
                                        cbS[:, s, :, :], ALU.mult)
                sq_t[s] = sqt

            def emit_var(s, local, last_local):
                for h in range(2):
                    nc.tensor.matmul(pvar[:, h, :HF], sel24[:, local, :12],
                                     sq_t[s][:, h, :],
                                     start=(local == 0),
                                     stop=(local == last_local))

            def emit_rstd(b):
                lnv = wk.tile([12, 2, HF], F32, tag="lnv", bufs=2, name="lnv")
                nc.scalar.activation(lnv[:], pvar[:, :, :HF],
                                     AF.Ln, bias=eps24[:12, :],
                                     scale=1.0 / D)
                rstd = wk.tile([12, 2, HF], BF16, tag="rstd", bufs=2,
                               name="rstd")
                nc.scalar.activation(rstd[:], lnv[:], AF.Exp, scale=-0.5)
                rstdw = wk.tile([12, 2, HF], BF16, tag="rstdw", bufs=2,
                                name="rstdw")
                nc.vector.tensor_scalar(rstdw[:], rstd[:],
                                        wcolb[b][:], None, ALU.mult)
                rstdw_t[b] = rstdw

            def passB_order(b):
                base = 12 * b
                order = []
                for k in range(TOPK):
                    for gl in range(2):
                        order.append(base + gl * TOPK + k)
                for j in range(KS):
                    for gl in range(2):
                        order.append(base + 8 + j * 2 + gl)
                return order

            def emit_passB(b, order):
                for s in order:
                    g, _, _ = SLOTS[s]
                    off = g * PAD_G
                    for h in range(2):
                        pA = ps.tile([128, HF], F32, tag="ph", name="pA",
                                     bufs=PS_BUFS["ph"])
                        nc.tensor.matmul(pA[:], wgm[:, s, :],
                                         rstdw_t[b][:, h, :],
                                         start=True, stop=True)
                        u = wk.tile([128, HF], F32, tag="u", bufs=4, name="u")
                        nc.vector.tensor_tensor(u[:], cbS[:, s, h, :], pA[:],
                                                ALU.mult)
                        asl = acc[:, off + h * HF:off + (h + 1) * HF]
                        if (off, h) not in first:
                            first.add((off, h))
                            nc.vector.tensor_tensor(
                                asl, u[:],
                                vembT[:, off + h * HF:off + (h + 1) * HF],
                                ALU.add)
                        else:
                            nc.vector.tensor_tensor(asl, asl, u[:], ALU.add)

            def emit_head(b):
                for g in (2 * b, 2 * b + 1):
                    off = g * PAD_G
                    asl = acc[:, off:off + PAD_G]
                    nc.vector.tensor_scalar(asl, asl, biasg[:, g:g + 1], None,
                                            ALU.add)
                    acc_bf = wk.tile([128, PAD_G], BF16, tag="accbf", bufs=2,
                                     name="acc_bf")
                    nc.vector.tensor_copy(acc_bf[:], asl)
                    for h in range(2):
                        pr = ps.tile([128, HF], F32, tag="ph", name="pr",
                                     bufs=PS_BUFS["ph"])
                        nc.tensor.matmul(pr[:], hW1[:],
                                         acc_bf[:, h * HF:(h + 1) * HF],
                                         start=True, stop=True)
                        r_bf = wk.tile([128, HF], BF16, tag="rbf", bufs=3,
                                       name="r_bf")
                        nc.scalar.activation(r_bf[:], pr[:], AF.Relu,
                                             bias=hb1[:])
                        po = ps.tile([128, HF], F32, tag="pc", name="po",
                                     bufs=PS_BUFS["pc"])
                        nc.tensor.matmul(po[:1, :], hW2[:], r_bf[:],
                                         start=True, stop=True)
                        nc.vector.tensor_scalar(
                            out_sb[:, off + h * HF:off + (h + 1) * HF],
                            po[:1, :], hb2[:], None, ALU.add)

            # batch 0 fronts
            for local in range(12):
                emit_front(local, local, 11)
            emit_var(11, 11, 11)
            emit_rstd(0)
            # batch 1 fronts, interleaved slot-by-slot with batch 0's pass B
            ord0 = passB_order(0)
            for local in range(12):
                emit_front(12 + local, local, 11)
                emit_passB(0, [ord0[local]])
            emit_var(23, 11, 11)
            emit_head(0)
            emit_rstd(1)
            emit_passB(1, passB_order(1))
            emit_head(1)

            nc.sync.dma_start(out_o[:], out_sb[:])

    nc.compile()
    return nc


# ------------------------------------------------------------------- driver

_CACHE = {}
LAST_RES = [None, None]


def kernel(**inputs):
    return _run(inputs, trace=False)[0]


def timed_run(inputs):
    _, t1, t2 = _run(inputs, trace=True)
    return t1, t2


def _run(inputs, trace=False):
    inp = {k: np.asarray(v) for k, v in inputs.items()}
    f32 = lambda k: inp[k].astype(np.float32)
    i64 = lambda k: inp[k].astype(np.int64)

    assert np.all(inp["be"] == 0), "nonzero be not supported"

    edge_cons, edge_vars, batch_idx = i64("edge_cons"), i64("edge_vars"), i64("batch_idx")
    plan = _plan(edge_cons, edge_vars, f32("edge_attr"), batch_idx)
    CW = tuple(plan["CW"])

    key1 = ("k1", CW)
    if key1 not in _CACHE:
        _CACHE[key1] = _build_k1(list(CW))
    nc1 = _CACHE[key1]

    P_bf = (np.eye(128) - 1.0 / 128).astype(np.float32).astype(BF)
    sel24 = _sel24()
    onesm = _onesm()

    c_feat = f32("c_feat")
    v_feat = f32("v_feat")
    counts = plan["counts"]
    ntot = plan["ntot"]

    dW2, sW2 = f32("dW2"), f32("sW2")
    W2all = np.ascontiguousarray(
        np.concatenate([dW2, sW2], axis=0).reshape(
            NE + KS, 4, 128, 128).transpose(3, 0, 1, 2)).astype(BF)
    b2allT = np.ascontiguousarray(
        np.concatenate([f32("db2"), f32("sb2")], axis=0).T).astype(BF)

    in1 = []
    for c in range(NCORE):
        nos = plan["node_of_slot"][c]
        vfT = np.zeros((VF, NC_NODES), np.float32)
        real = nos >= 0
        vfT[:, real] = v_feat[nos[real]].T
        cnt = counts[c].astype(np.float32)
        padc = (PAD_G - counts[c]).astype(np.float32)
        ecidx = plan["ecidx"][c]
        used = plan["used"][c]
        cfa = np.zeros((128 * ntot, CF1), np.float32)
        cfa[used, :CF] = c_feat[ecidx[used]]
        cfa[used, CF] = 1.0
        m = dict(
            ecf=np.ascontiguousarray(
                cfa.reshape(ntot, 128, CF1).transpose(1, 0, 2).reshape(
                    128, ntot * CF1)).astype(BF),
            oea=_build_oea(plan, c),
            Wca=np.concatenate([f32("Wc"), f32("bc").reshape(1, D)],
                               axis=0).astype(BF),
            Wv=f32("Wv"), bv_col=f32("bv").reshape(D, 1),
            vfeatT=vfT,
            We_col=f32("We").reshape(D, 1),
            lng_col=f32("ln_g").reshape(D, 1), lnb_col=f32("ln_b").reshape(D, 1),
            P_bf=P_bf,
            WqT=np.ascontiguousarray(f32("Wq").T),
            tokKT=np.ascontiguousarray(f32("tokK").T),
            bq_col=f32("bq").reshape(TD, 1),
            tokV=f32("tokV").astype(BF),
            Wg_r=np.ascontiguousarray(f32("Wg").reshape(2, D, NE).transpose(1, 0, 2)),
            bg_col=f32("bg").reshape(NE, 1), eb_col=f32("ebias").reshape(NE, 1),
            alpha11=f32("alpha").reshape(1, 1).astype(BF),
            sel24=sel24, onesm=onesm,
            padc4=np.tile(padc[None, :], (128, 1)),
            invc4=np.tile((1.0 / np.maximum(cnt, 1.0))[None, :], (128, 1)),
            negpadc=(-padc).reshape(1, GPC).astype(BF),
            W2all=W2all, b2allT=b2allT,
        )
        in1.append(m)

    res1 = run_bass_kernel_spmd(nc1, in1, CORE_IDS, trace=trace)
    LAST_RES[0] = res1

    explog = np.concatenate(
        [np.asarray(res1.results[c]["explogT"]).T.astype(np.float32)
         for c in range(NCORE)], axis=0)                          # [B, NE]
    top_idx = np.argsort(-explog, axis=1, kind="stable")[:, :TOPK]  # [B, 4]
    mask = np.zeros((B, NE), np.float32)
    np.put_along_axis(mask, top_idx, 1.0, axis=1)

    if "k2" not in _CACHE:
        _CACHE["k2"] = _build_k2()
    nc2 = _CACHE["k2"]

    dW1 = f32("dW1")
    dg, dbb = f32("dg"), f32("dbb")
    sW1 = f32("sW1")
    sg, sbb = f32("sg"), f32("sbb")

    shifts_c = np.zeros((24, 2, 12), np.float32)
    for b in range(2):
        for i in range(12):
            shifts_c[12 * b + i, b, i] = 1.0
    shifts_c = shifts_c.reshape(24, 2 * 12).astype(BF)
    in2 = []
    for c in range(NCORE):
        # dedicated experts in packed (batch-major) slot order
        sel = np.array([top_idx[c * GPC + g, k] for g, k in DED_GK])  # [16]
        Esel24 = np.zeros((24, NE), np.float32)
        Gsel24 = np.zeros((GPC, 24), np.float32)
        sh05 = np.zeros((24, 1), np.float32)
        gmask24 = np.zeros((24, GPC), np.float32)
        bb24 = np.zeros((24, D), np.float32)
        wgm = np.zeros((12, NCH, 128), np.float32)
        nded = 0
        for s, (g, wi, b1i) in enumerate(SLOTS):
            gmask24[s, g] = 1.0
            if b1i >= 0:
                e = sel[nded]; nded += 1
                Esel24[s, e] = 1.0
                Gsel24[g, s] = 1.0
                bb24[s] = dbb[e]
                wgm[s % 12, s, :] = dg[e]
            else:
                j = -1 - b1i
                sh05[s, 0] = 1.0 / KS
                bb24[s] = sbb[j]
                wgm[s % 12, s, :] = sg[j]
        W1s = dW1[sel]                                  # [16, 128, 512]
        b1s = f32("db1")[sel]                           # [16, 512]
        W2Pall = np.asarray(res1.results[c]["W2Pall"]).reshape(D, NE + KS,
                                                               4, 128)
        b2Pall = np.asarray(res1.results[c]["b2Pall"]).astype(np.float32)
        slotmap = np.concatenate([sel, NE + np.arange(KS)])
        W2Psel = np.ascontiguousarray(W2Pall[:, slotmap])
        b2Psel = np.ascontiguousarray(b2Pall[:, slotmap])
        m = dict(
            vembT_bf=np.asarray(res1.results[c]["vembT"]).astype(BF),
            explog_nm=explog[c * GPC:(c + 1) * GPC],
            mask_nm=mask[c * GPC:(c + 1) * GPC],
            Esel24=Esel24, Gsel24=Gsel24, sh05=sh05,
            W1sel=np.ascontiguousarray(W1s.transpose(1, 0, 2)).astype(BF),
            sW1T=np.ascontiguousarray(sW1.transpose(1, 0, 2)).astype(BF),
            b1selT=np.ascontiguousarray(
                b1s.reshape(NSLOT, 4, 128).transpose(2, 0, 1).reshape(
                    128, NSLOT * 4)),
            sb1T=np.ascontiguousarray(
                f32("sb1").reshape(KS, 4, 128).transpose(2, 0, 1).reshape(
                    128, KS * 4)),
            W2Psel=W2Psel, b2Psel=b2Psel,
            wgm=wgm.reshape(12, NCH * 128).astype(BF),
            sel24=sel24, shifts=shifts_c,
            bb24=bb24.astype(BF),
            gmask24=gmask24,
            hW1=f32("hW1").astype(BF), hb1_col=f32("hb1").reshape(D, 1),
            hW2col=f32("hW2").reshape(D, 1).astype(BF),
            hb2=f32("hb2").reshape(1, 1),
        )
        in2.append(m)

    res2 = run_bass_kernel_spmd(nc2, in2, CORE_IDS, trace=trace)
    LAST_RES[1] = res2

    out = np.zeros(N, np.float32)
    for c in range(NCORE):
        row = np.asarray(res2.results[c]["out_row"],
                         dtype=np.float32).reshape(-1)
        nos = plan["node_of_slot"][c]
        real = nos >= 0
        out[nos[real]] = row[real]
    return out, res1.exec_time_ns, res2.exec_time_ns


# revision 49
# speedup vs baseline: 1.0261x; 1.0194x over previous
"""Trainium2 Bass kernel for nn_MoEPolicy (moe_routing).

Strategy (8 NeuronCores, SPMD, no collectives):
  - 32 graphs -> 4 graphs per core; each graph padded to 768 node slots
    (3072 padded node slots per core, 24 windows of 128).
  - Kernel 1 (per core): edge aggregation via one-hot PSUM matmuls (bf16
    one-hot scaled by edge_attr), v_emb (relu+LN), struct-token attention
    (batched, no per-node softmax max-subtract: scores are < 0.02 in
    magnitude), masked pooling, gating logits.  All heavy elementwise work
    batched into [128, 512] group ops; single activation table set
    (Ln/Exp/Relu/Square) -> one table load.
  - Host: top-4 expert selection per graph from device logits (index
    selection only), slices expert weights per core.
  - Kernel 2 (per core): route weights on device, two-pass expert
    pipeline: pass A computes all 24 expert chunk outputs (gelu on the
    scalar engine, bf16 matmuls), variances batched into one [24, 768]
    PSUM tile via selector-matmuls; one Ln+Exp gives all rstd rows; pass B
    broadcasts rstd*(route weight) via masked rank-24 matmuls and
    accumulates into the residual; task head.
All floating-point model math runs on device; the host only shards, pads,
permutes, selects indices, and casts dtypes.
"""

import sys

for _p in ("/opt/trn_rl_repo",):
    if _p not in sys.path:
        sys.path.insert(0, _p)

import numpy as np
import ml_dtypes

import concourse.bacc as bacc
import concourse.mybir as mybir
import concourse.tile as tile
from concourse.bass_utils import run_bass_kernel_spmd

F32 = mybir.dt.float32
F32R = mybir.dt.float32r
BF16 = mybir.dt.bfloat16
AF = mybir.ActivationFunctionType
ALU = mybir.AluOpType
AX = mybir.AxisListType
BF = ml_dtypes.bfloat16

# problem constants
D = 128
TD = 128
T = 64
NE = 16
KS = 2
TOPK = 4
TEMP = 0.6
B = 32
M = 10000
N = 20000
E = 160000
CF, VF, EF = 4, 6, 1

NCORE = 8
GPC = B // NCORE            # graphs per core
PAD_G = 768                 # node slots per graph
NC_NODES = GPC * PAD_G      # 3072
WPG = PAD_G // 128          # windows per graph (6)
NWIN = GPC * WPG            # 24 windows per core
NGRP = NWIN // 4            # 6 groups of 4 windows
LN_EPS = 1e-5
ISQ_TD = 1.0 / float(np.sqrt(np.float32(TD)))
CF1 = CF + 1

NSLOT = GPC * TOPK          # 16 dedicated (graph, k) slots per core
NCH = NSLOT + KS * GPC      # 24 chunk-slots (16 ded + 2 shared x 4 graphs)
HF = PAD_G // 2             # 384

CORE_IDS = list(range(NCORE))


# ---------------------------------------------------------------- host plan

def _plan(edge_cons, edge_vars, edge_attr, batch_idx):
    """Node slot assignment + edge window schedule. Pure index work."""
    order = np.argsort(batch_idx, kind="stable")
    bs = batch_idx[order]
    deg = np.bincount(edge_vars, minlength=N)

    node_of_slot = -np.ones((NCORE, NC_NODES), dtype=np.int64)
    slot_of_node = np.empty(N, dtype=np.int64)       # global slot = core*NC + s
    counts = np.zeros((NCORE, GPC), dtype=np.int64)  # real nodes per graph

    for g in range(B):
        nodes = order[np.searchsorted(bs, g, side="left"):
                      np.searchsorted(bs, g, side="right")]
        core, lg = g // GPC, g % GPC
        counts[core, lg] = len(nodes)
        if len(nodes) > PAD_G:
            raise RuntimeError(f"graph {g} has {len(nodes)} nodes > PAD_G={PAD_G}")
        # balance edge load across the graph's WPG windows
        nds = nodes[np.argsort(-deg[nodes], kind="stable")]
        wload = np.zeros(WPG, dtype=np.int64)
        wfill = np.zeros(WPG, dtype=np.int64)
        base = lg * PAD_G
        for nd in nds:
            cand = np.where(wfill < 128)[0]
            w = cand[np.argmin(wload[cand])]
            s = base + w * 128 + wfill[w]
            node_of_slot[core, s] = nd
            slot_of_node[nd] = core * NC_NODES + s
            wload[w] += deg[nd]
            wfill[w] += 1

    # edges -> (core, window, lane j)
    eslot = slot_of_node[edge_vars]
    ecore = eslot // NC_NODES
    es = eslot % NC_NODES
    ewin = es // 128
    ej = es % 128

    # tiles per window position, shared across cores
    cw = np.zeros((NCORE, NWIN), dtype=np.int64)
    per = {}
    for c in range(NCORE):
        sel = np.where(ecore == c)[0]
        for w in range(NWIN):
            ews = sel[ewin[sel] == w]
            per[(c, w)] = ews
            cw[c, w] = max(1, -(-len(ews) // 128))
    CW = cw.max(axis=0)
    ntot = int(CW.sum())

    ecidx = np.zeros((NCORE, 128 * ntot), dtype=np.int64)   # cons index per slot
    used = np.zeros((NCORE, 128 * ntot), dtype=bool)
    vloc = np.full((NCORE, 128 * ntot), -1.0, dtype=np.float32)
    eav = np.zeros((NCORE, 128 * ntot), dtype=np.float32)
    offs = np.concatenate([[0], np.cumsum(CW)]) * 128
    ea_flat = edge_attr.reshape(-1).astype(np.float32)
    for c in range(NCORE):
        for w in range(NWIN):
            ews = per[(c, w)]
            o = offs[w]
            ecidx[c, o:o + len(ews)] = edge_cons[ews]
            used[c, o:o + len(ews)] = True
            vloc[c, o:o + len(ews)] = ej[ews]
            eav[c, o:o + len(ews)] = ea_flat[ews]

    return dict(node_of_slot=node_of_slot, counts=counts, CW=CW.tolist(),
                ntot=ntot, ecidx=ecidx, used=used, vloc=vloc, eav=eav)


def _build_oea(plan, c):
    """One-hot (scaled by edge_attr) [128 lanes, tile, 128 nodes], bf16."""
    ntot = plan["ntot"]
    vloc = plan["vloc"][c].reshape(ntot, 128)
    eav = plan["eav"][c].reshape(ntot, 128)
    arr = np.zeros((128, ntot, 128), np.float32)   # [lane, tile, n]
    t_i, p_i = np.nonzero(vloc >= 0)
    arr[p_i, t_i, vloc[t_i, p_i].astype(np.int64)] = eav[t_i, p_i]
    return np.ascontiguousarray(arr.reshape(128, ntot * 128)).astype(BF)


def _sel24():
    """[128, 24, 24] bf16: SEL24[:, w, j] = (j == w)."""
    s = np.zeros((128, 24, 24), np.float32)
    for w in range(24):
        s[:, w, w] = 1.0
    return s.reshape(128, 24 * 24).astype(BF)


def _onesm():
    """[24, 24, 128] bf16: ONESM[r, w, :] = (r == w)."""
    s = np.zeros((24, 24, 128), np.float32)
    for w in range(24):
        s[w, w, :] = 1.0
    return s.reshape(24, 24 * 128).astype(BF)




# two batches: batch b covers graphs {2b, 2b+1}; 8 dedicated + 4 shared each.
# slot s order: [b0: ded g0k0..g1k3, sh j0g0, j0g1, j1g0, j1g1] then batch 1.
def _slots():
    out = []   # per slot: (graph, wi, b1idx)  wi: index into W2Psel/b2Psel
    nded = 0
    for b in range(2):
        for g in (2 * b, 2 * b + 1):
            for k in range(TOPK):
                out.append((g, nded, nded))
                nded += 1
        for j in range(KS):
            for g in (2 * b, 2 * b + 1):
                out.append((g, NSLOT + j, -1 - j))
    return out


SLOTS = _slots()
DED_GK = []   # (graph, k) in packed ded order
for b in range(2):
    for g in (2 * b, 2 * b + 1):
        for k in range(TOPK):
            DED_GK.append((g, k))

# ------------------------------------------------------------- build kernel1

DEBUG_K1 = False


def _build_k1(CW):
    ntot = int(sum(CW))
    nc = bacc.Bacc("TRN2", target_bir_lowering=False, debug=False,
                   num_devices=NCORE)

    def din(name, shape, dt=F32):
        return nc.dram_tensor(name, shape, dt, kind="ExternalInput")

    ecf_i = din("ecf", [128, ntot * CF1], BF16)
    oea_i = din("oea", [128, ntot * 128], BF16)
    Wca_i = din("Wca", [CF1, D], BF16)
    Wv_i = din("Wv", [VF, D])
    bv_i = din("bv_col", [D, 1])
    vfT_i = din("vfeatT", [VF, NC_NODES])
    We_i = din("We_col", [D, 1])
    lng_i = din("lng_col", [D, 1])
    lnb_i = din("lnb_col", [D, 1])
    P_i = din("P_bf", [128, 128], BF16)
    WqT_i = din("WqT", [TD, D])
    tokKT_i = din("tokKT", [TD, T])
    bq_i = din("bq_col", [TD, 1])
    tokV_i = din("tokV", [T, TD], BF16)
    Wg_i = din("Wg_r", [D, 2, NE])
    bg_i = din("bg_col", [NE, 1])
    eb_i = din("eb_col", [NE, 1])
    al_i = din("alpha11", [1, 1], BF16)
    sel24_i = din("sel24", [128, 24 * 24], BF16)
    onesm_i = din("onesm", [24, 24 * 128], BF16)
    padc4_i = din("padc4", [128, GPC])
    invc4_i = din("invc4", [128, GPC])
    negpadc_i = din("negpadc", [1, GPC], BF16)
    W2a_i = din("W2all", [D, NSLOT + KS, 4, 128], BF16)
    b2a_i = din("b2allT", [D, NSLOT + KS], BF16)

    vembT_o = nc.dram_tensor("vembT", [D, NC_NODES], BF16, kind="ExternalOutput")
    exlg_o = nc.dram_tensor("explogT", [NE, GPC], F32, kind="ExternalOutput")
    W2P_o = nc.dram_tensor("W2Pall", [D, (NSLOT + KS) * 4 * 128], BF16,
                           kind="ExternalOutput")
    b2P_o = nc.dram_tensor("b2Pall", [D, NSLOT + KS], F32,
                           kind="ExternalOutput")

    offs = np.concatenate([[0], np.cumsum(CW)]).astype(int)
    goffs = [int(offs[4 * g]) for g in range(NGRP + 1)]   # tile offsets per group

    with tile.TileContext(nc) as tc:
        with (
            tc.tile_pool(name="cp", bufs=1) as cp,
            tc.tile_pool(name="oh", bufs=2) as ohp,
            tc.tile_pool(name="wk", bufs=3) as wk,
            tc.tile_pool(name="sm", bufs=4) as smp,
            tc.tile_pool(name="ps", bufs=1, space="PSUM") as ps,
        ):
            PS_BUFS = {"g1": 2, "mm": 3, "pa": 2}
            _ld = [0]
            def load(ap_dram, shape, dt=F32):
                _ld[0] += 1
                t_ = cp.tile(shape, dt, tag=f"cst{_ld[0]}", name=f"cst{_ld[0]}")
                src_ap = ap_dram[:]
                if dt == F32R:
                    src_ap = src_ap.bitcast(F32R)
                nc.sync.dma_start(t_[:], src_ap)
                return t_

            ecf_s = load(ecf_i, [128, ntot * CF1], BF16)
            # group 0/1 one-hot DMAs first: they head the critical path
            oea_pre = []
            for _g in range(2):
                gt0, gt1 = goffs[_g], goffs[_g + 1]
                _t = ohp.tile([128, 32 * 128], BF16, tag="oea", name="oeaw")
                nc.sync.dma_start(_t[:, :(gt1 - gt0) * 128],
                                  oea_i[:, gt0 * 128:gt1 * 128])
                oea_pre.append(_t)
            Wca_s = load(Wca_i, [CF1, D], BF16)
            Wv_s = load(Wv_i, [VF, D], F32R)
            bv_s = load(bv_i, [D, 1])
            vfT_s = load(vfT_i, [VF, NC_NODES], F32R)
            We_s = load(We_i, [D, 1])
            lng_s = load(lng_i, [D, 1])
            lnb_s = load(lnb_i, [D, 1])
            P_s = load(P_i, [128, 128], BF16)
            WqT_s = load(WqT_i, [TD, D], F32R)
            tKT_s = load(tokKT_i, [TD, T], F32R)
            bq_s = load(bq_i, [TD, 1], F32R)
            tV_s = load(tokV_i, [T, TD], BF16)
            Wg_s = load(Wg_i, [D, 2, NE], F32R)
            bg_s = load(bg_i, [NE, 1])
            eb_s = load(eb_i, [NE, 1])
            al_s = load(al_i, [1, 1], BF16)
            sel24 = load(sel24_i, [128, 24, 24], BF16)
            onesm = load(onesm_i, [24, 24, 128], BF16)
            padc4 = load(padc4_i, [128, GPC])
            invc4 = load(invc4_i, [128, GPC])
            negpadc = load(negpadc_i, [1, GPC], BF16)

            onesr_bf = cp.tile([1, 128], BF16, name="onesr_bf")
            nc.vector.memset(onesr_bf[:], 1.0)
            onesc_bf = cp.tile([128, 1], BF16, name="onesc_bf")
            nc.vector.memset(onesc_bf[:], 1.0)
            eps24 = cp.tile([24, 1], F32, name="eps24")
            nc.vector.memset(eps24[:], LN_EPS)

            # persistent big tiles
            c_all = cp.tile([128, NGRP, 4, 128], F32, name="c_all")
            v0b_all = cp.tile([128, NGRP, 512], F32, name="v0b_all")
            vembT_s = cp.tile([128, NWIN, 128], BF16, name="vembT_s")
            wsum = cp.tile([128, NWIN], F32, name="wsum")
            varsb = cp.tile([24, NGRP, 128], F32, name="varsb")
            rstd24 = cp.tile([24, NGRP, 128], BF16, name="rstd24")
            Wp_s = cp.tile([D, T], BF16, name="Wp_s")       # Wq @ tokK^T
            bqK_s = cp.tile([1, T], BF16, name="bqK_s")

            # ---- prologue: W' = Wq @ tokK^T  [D, T]; bqK = bq^T tokK^T
            pWp = ps.tile([128, 512], F32, tag="mm", name="pWp",
                          bufs=PS_BUFS["mm"])
            nc.tensor.matmul(pWp[:, :T], WqT_s[:], tKT_s[:], start=True, stop=True)
            nc.vector.tensor_copy(Wp_s[:], pWp[:, :T])
            pbq = ps.tile([NE, 512], F32, tag="g1", name="pbq",
                          bufs=PS_BUFS["g1"])
            nc.tensor.matmul(pbq[:1, :T], bq_s[:], tKT_s[:], start=True, stop=True)
            nc.vector.tensor_copy(bqK_s[:], pbq[:1, :T])

            # ---- v0 for all groups up front (independent of edges)
            for grp in range(NGRP):
                pv0 = ps.tile([128, 512], F32, tag="mm", name="pv0",
                              bufs=PS_BUFS["mm"])
                nc.tensor.matmul(pv0[:], Wv_s[:],
                                 vfT_s[:, grp * 512:(grp + 1) * 512],
                                 start=True, stop=True)
                nc.vector.tensor_scalar(v0b_all[:, grp, :], pv0[:], bv_s[:],
                                        None, ALU.add)

            # ---- pad-column head: x=relu(bv); c=P x; var -> varsb[0, 5, 0]
            z0 = smp.tile([128, 1], F32, tag="pad", name="z0")
            nc.vector.memset(z0[:], 0.0)
            xp = smp.tile([128, 1], BF16, tag="padb", name="xp")
            nc.scalar.activation(xp[:], z0[:], AF.Relu, bias=bv_s[:])
            pcp = ps.tile([128, 512], F32, tag="mm", name="pcp",
                          bufs=PS_BUFS["mm"])
            nc.tensor.matmul(pcp[:, :1], P_s[:], xp[:], start=True, stop=True)
            cgp = smp.tile([128, 1], F32, tag="pad", name="cgp")
            nc.vector.tensor_scalar(cgp[:], pcp[:, :1], lng_s[:], None, ALU.mult)
            sqp = smp.tile([128, 1], BF16, tag="padb", name="sqp")
            nc.vector.tensor_tensor(sqp[:], cgp[:], cgp[:], ALU.mult)
            pvp = ps.tile([NE, 512], F32, tag="g1", name="pvp",
                          bufs=PS_BUFS["g1"])
            nc.tensor.matmul(pvp[:1, :1], onesc_bf[:], sqp[:], start=True, stop=True)
            nc.vector.tensor_copy(varsb[0:1, NGRP - 1:NGRP, 0:1], pvp[:1, :1])

            # ---- phase 1, software pipelined: G1(g) | midA(g-1) | midB(g-2)
            def midA(grp):
                pT1 = ps.tile([128, 512], F32, tag="mm", name="pT1",
                              bufs=PS_BUFS["mm"])
                nc.tensor.matmul(pT1[:], Wca_s[:], G1t[grp][:],
                                 start=True, stop=True)
                s_sb = wk.tile([128, 512], F32, tag="s", name="s_sb")
                nc.vector.scalar_tensor_tensor(
                    s_sb[:], pT1[:], We_s[:], v0b_all[:, grp, :],
                    ALU.mult, ALU.add)
                x_bf = wk.tile([128, 512], BF16, tag="x", name="x_bf")
                nc.scalar.activation(x_bf[:], s_sb[:], AF.Relu)
                pc = ps.tile([128, 512], F32, tag="mm", name="pc",
                             bufs=PS_BUFS["mm"])
                nc.tensor.matmul(pc[:], P_s[:], x_bf[:], start=True, stop=True)
                nc.vector.tensor_scalar(
                    c_all[:, grp, :, :], pc[:], lng_s[:], None, ALU.mult)
                sqt = wk.tile([128, 4, 128], BF16, tag="sq", name="sqt")
                nc.vector.tensor_tensor(sqt[:], c_all[:, grp, :, :],
                                        c_all[:, grp, :, :], ALU.mult)
                sq_t[grp] = sqt

            def midB(grp):
                pvarg = ps.tile([24, 128], F32, tag="g1", name="pvarg",
                                bufs=PS_BUFS["g1"])
                for wi in range(4):
                    w = grp * 4 + wi
                    nc.tensor.matmul(pvarg[:], sel24[:, w, :],
                                     sq_t[grp][:, wi, :],
                                     start=(wi == 0), stop=(wi == 3))
                nc.vector.tensor_copy(varsb[:, grp, :], pvarg[:])

            G1t = [None] * NGRP
            sq_t = [None] * NGRP
            for grp in range(NGRP):
                gt0, gt1 = goffs[grp], goffs[grp + 1]
                nt = gt1 - gt0
                if grp < 2:
                    oeaw = oea_pre[grp]
                else:
                    oeaw = ohp.tile([128, 32 * 128], BF16, tag="oea",
                                    name="oeaw")
                    nc.sync.dma_start(oeaw[:, :nt * 128],
                                      oea_i[:, gt0 * 128:gt1 * 128])
                pG1 = ps.tile([5, 512], F32, tag="g1", name="pG1",
                              bufs=PS_BUFS["g1"])
                for wi in range(4):
                    w = grp * 4 + wi
                    for t_ in range(int(CW[w])):
                        gt = int(offs[w]) + t_
                        lt = gt - gt0
                        nc.tensor.matmul(
                            pG1[:CF1, wi * 128:(wi + 1) * 128],
                            ecf_s[:, gt * CF1:(gt + 1) * CF1],
                            oeaw[:, lt * 128:(lt + 1) * 128],
                            start=(t_ == 0), stop=(t_ == int(CW[w]) - 1))
                G1sb = wk.tile([CF1, 512], BF16, tag="g1sb", bufs=2, name="G1sb")
                nc.vector.tensor_copy(G1sb[:], pG1[:CF1, :])
                G1t[grp] = G1sb
                if grp >= 1:
                    midA(grp - 1)
                if grp >= 2:
                    midB(grp - 2)
            midA(NGRP - 1)
            midB(NGRP - 2)
            midB(NGRP - 1)

            # W2 fold inputs: issue DMA now so it rides behind the oea loads
            W2a_s = cp.tile([D, NSLOT + KS, 4, 128], BF16, name="W2a_s")
            nc.sync.dma_start(W2a_s[:], W2a_i[:])
            b2a_s = cp.tile([D, NSLOT + KS], BF16, name="b2a_s")
            nc.sync.dma_start(b2a_s[:], b2a_i[:])

            # ---- rstd for all windows (incl pad at [0, NGRP-1, 0])
            lnv = wk.tile([24, NGRP, 128], F32, tag="lnv", bufs=1, name="lnv")
            nc.scalar.activation(lnv[:], varsb[:], AF.Ln,
                                 bias=eps24[:], scale=1.0 / D)
            nc.scalar.activation(rstd24[:], lnv[:], AF.Exp, scale=-0.5)

            # ---- pad-column tail (uses batched pad rstd)
            pbb = ps.tile([128, 512], F32, tag="mm", name="pbb",
                          bufs=PS_BUFS["mm"])
            nc.tensor.matmul(pbb[:, :1], onesr_bf[:],
                             rstd24[0:1, NGRP - 1, 0:1], start=True, stop=True)
            up = smp.tile([128, 1], F32, tag="pad", name="up")
            nc.vector.tensor_tensor(up[:], cgp[:], pbb[:, :1], ALU.mult)
            vp = smp.tile([128, 1], BF16, tag="padb", name="vp")
            nc.vector.tensor_scalar(vp[:], up[:], lnb_s[:], None, ALU.add)
            pscp = ps.tile([NE, 512], F32, tag="g1", name="pscp",
                           bufs=PS_BUFS["g1"])
            nc.tensor.matmul(pscp[:1, :T], vp[:], Wp_s[:], start=True, stop=False)
            nc.tensor.matmul(pscp[:1, :T], onesr_bf[:, :1], bqK_s[:],
                             start=False, stop=True)
            exps = smp.tile([1, T], F32, tag="padr", name="exps")
            nc.scalar.activation(exps[:], pscp[:1, :T], AF.Exp, scale=ISQ_TD)
            smsum = smp.tile([1, 1], F32, tag="pads", name="smsum")
            nc.vector.tensor_reduce(smsum[:], exps[:], AX.X, ALU.add)
            rcp = smp.tile([1, 1], F32, tag="pads", name="rcp")
            nc.vector.reciprocal(rcp[:], smsum[:])
            wtsp = smp.tile([1, T], BF16, tag="padr", name="wtsp")
            nc.vector.tensor_scalar(wtsp[:], exps[:], rcp[:], None, ALU.mult)

            # ---- phase 2 + struct scores, software pipelined per group
            R = ps.tile([64, 8], F32, tag="g1", name="R", bufs=PS_BUFS["g1"])

            def rowsums(grp):
                for wi in range(4):
                    w = grp * 4 + wi
                    g, j = w // WPG, w % WPG
                    nc.tensor.matmul(R[:T, g:g + 1], wts_t[grp][:, wi, :],
                                     onesc_bf[:], start=(j == 0),
                                     stop=(j == WPG - 1))

            wts_t = [None] * NGRP
            for grp in range(NGRP):
                pA = ps.tile([128, 4, 128], F32, tag="pa", name="pA",
                             bufs=PS_BUFS["pa"])
                for wi in range(4):
                    w = grp * 4 + wi
                    nc.tensor.matmul(pA[:, wi, :], onesm[:, w, :],
                                     rstd24[:, grp, :], start=True, stop=True)
                u_sb = wk.tile([128, 4, 128], F32, tag="u", name="u_sb")
                nc.vector.tensor_tensor(u_sb[:], c_all[:, grp, :, :], pA[:],
                                        ALU.mult)
                nc.scalar.activation(vembT_s[:, 4 * grp:4 * grp + 4, :],
                                      u_sb[:], AF.Identity, bias=lnb_s[:])
                nc.vector.tensor_reduce(wsum[:, 4 * grp:4 * grp + 4],
                                        u_sb[:], AX.X, ALU.add)
                psc = ps.tile([128, 4, 64], F32, tag="pa", name="psc",
                              bufs=PS_BUFS["pa"])
                for wi in range(4):
                    w = grp * 4 + wi
                    nc.tensor.matmul(psc[:, wi, :], vembT_s[:, w, :], Wp_s[:],
                                     start=True, stop=False)
                    nc.tensor.matmul(psc[:, wi, :], onesr_bf[:], bqK_s[:],
                                     start=False, stop=True)
                ex = wk.tile([128, 4, 64], BF16, tag="ex", bufs=2, name="ex")
                nc.scalar.activation(ex[:], psc[:], AF.Exp, scale=ISQ_TD)
                sme = smp.tile([128, 4], F32, tag="sme", bufs=3, name="sme")
                nc.vector.tensor_reduce(sme[:], ex[:], AX.X, ALU.add)
                rce = smp.tile([128, 4], F32, tag="rce", bufs=3, name="rce")
                nc.vector.reciprocal(rce[:], sme[:])
                wts = wk.tile([128, 4, 64], BF16, tag="wts", bufs=3, name="wts")
                for wi in range(4):
                    nc.vector.tensor_scalar(wts[:, wi, :], ex[:, wi, :],
                                            rce[:, wi:wi + 1], None, ALU.mult)
                wts_t[grp] = wts
                if grp >= 1:
                    rowsums(grp - 1)
            rowsums(NGRP - 1)
            nc.tensor.matmul(R[:T, GPC:2 * GPC], wtsp[:], negpadc[:],
                             start=True, stop=True)

            nc.sync.dma_start(vembT_o[:], vembT_s[:])

            # ---- struct pooling
            Rsb = smp.tile([64, 2 * GPC], F32, tag="Rsb", bufs=1, name="Rsb")
            nc.vector.tensor_copy(Rsb[:], R[:T, :2 * GPC])
            Rc = smp.tile([64, GPC], BF16, tag="Rc", bufs=1, name="Rc")
            nc.vector.tensor_tensor(Rc[:], Rsb[:, :GPC], Rsb[:, GPC:2 * GPC],
                                    ALU.add)
            pstr = ps.tile([128, 512], F32, tag="mm", name="pstr",
                           bufs=PS_BUFS["mm"])
            nc.tensor.matmul(pstr[:, :GPC], tV_s[:], Rc[:], start=True, stop=True)
            strT = smp.tile([128, GPC], F32R, tag="strT", bufs=1, name="strT")
            with nc.allow_low_precision(reason="gating rhs f32r"):
                nc.vector.tensor_tensor(strT[:], pstr[:, :GPC], invc4[:],
                                        ALU.mult)

            # ---- graph embedding pooling with pad correction
            gsum = smp.tile([128, GPC], F32, tag="gsum", bufs=1, name="gsum")
            for g in range(GPC):
                nc.vector.tensor_reduce(gsum[:, g:g + 1],
                                        wsum[:, g * WPG:(g + 1) * WPG],
                                        AX.X, ALU.add)
            t3 = smp.tile([128, GPC], F32, tag="t3", bufs=1, name="t3")
            nc.vector.tensor_scalar(t3[:], padc4[:], up[:], None, ALU.mult)
            t4 = smp.tile([128, GPC], F32, tag="t4", bufs=1, name="t4")
            nc.vector.tensor_tensor(t4[:], gsum[:], t3[:], ALU.subtract)
            t5 = smp.tile([128, GPC], F32, tag="t5", bufs=1, name="t5")
            nc.vector.tensor_tensor(t5[:], t4[:], invc4[:], ALU.mult)
            gembT = smp.tile([128, GPC], F32R, tag="gembT", bufs=1, name="gembT")
            with nc.allow_low_precision(reason="gating rhs f32r"):
                nc.vector.tensor_scalar(gembT[:], t5[:], lnb_s[:], None, ALU.add)

            # ---- gating logits -> exp(logits)
            pl = ps.tile([NE, 512], F32, tag="g1", name="pl", bufs=PS_BUFS["g1"])
            nc.tensor.matmul(pl[:, :GPC], Wg_s[:, 0, :], gembT[:],
                             start=True, stop=False)
            nc.tensor.matmul(pl[:, :GPC], Wg_s[:, 1, :], strT[:],
                             start=False, stop=True)
            pa_ = ps.tile([128, 512], F32, tag="mm", name="pa_",
                          bufs=PS_BUFS["mm"])
            nc.tensor.matmul(pa_[:NE, :1], onesr_bf[:, :NE], al_s[:],
                             start=True, stop=True)
            acol = smp.tile([NE, 1], F32, tag="acol", bufs=1, name="acol")
            nc.vector.tensor_copy(acol[:], pa_[:NE, :1])
            lg1 = smp.tile([NE, GPC], F32, tag="lg1", bufs=1, name="lg1")
            nc.vector.tensor_scalar(lg1[:], pl[:, :GPC], bg_s[:], None, ALU.add)
            lg2 = smp.tile([NE, GPC], F32, tag="lg2", bufs=1, name="lg2")
            nc.vector.tensor_scalar(lg2[:], lg1[:], acol[:], 1.0 / TEMP,
                                    ALU.mult, ALU.mult)
            lg3 = smp.tile([NE, GPC], F32, tag="lg3", bufs=1, name="lg3")
            nc.vector.tensor_scalar(lg3[:], lg2[:], eb_s[:], None, ALU.add)
            exlg = smp.tile([NE, GPC], F32, tag="exlg", bufs=1, name="exlg")
            nc.scalar.activation(exlg[:], lg3[:], AF.Exp)
            nc.sync.dma_start(exlg_o[:], exlg[:])

            # ---- W2 fold for all experts: W2P = (W2_chunk @ P), h-major
            W2P = cp.tile([128, NSLOT + KS, 4, 128], BF16, name="W2P")
            for s in range(NSLOT + KS):
                pw = ps.tile([128, 512], F32, tag="mm", name="pw",
                             bufs=PS_BUFS["mm"])
                for c in range(4):
                    nc.tensor.matmul(pw[:, c * 128:(c + 1) * 128],
                                     W2a_s[:, s, c, :], P_s[:],
                                     start=True, stop=True)
                nc.scalar.copy(W2P[:, s, :, :], pw[:])
                nc.sync.dma_start(W2P_o[:, s * 512:(s + 1) * 512],
                                  W2P[:, s, :, :])
            pb2 = ps.tile([128, 512], F32, tag="mm", name="pb2",
                          bufs=PS_BUFS["mm"])
            nc.tensor.matmul(pb2[:, :NSLOT + KS], P_s[:], b2a_s[:],
                             start=True, stop=True)
            b2P = cp.tile([D, NSLOT + KS], F32, name="b2P")
            nc.vector.tensor_copy(b2P[:], pb2[:, :NSLOT + KS])
            nc.sync.dma_start(b2P_o[:], b2P[:])

    nc.compile()
    return nc


# ------------------------------------------------------------- build kernel2

def _build_k2():
    nc = bacc.Bacc("TRN2", target_bir_lowering=False, debug=False,
                   num_devices=NCORE)

    def din(name, shape, dt=F32):
        return nc.dram_tensor(name, shape, dt, kind="ExternalInput")

    vembT_i = din("vembT_bf", [D, NC_NODES], BF16)
    explog_i = din("explog_nm", [GPC, NE])
    mask_i = din("mask_nm", [GPC, NE])
    Esel_i = din("Esel24", [24, NE])
    Gsel_i = din("Gsel24", [GPC, 24])
    sh05_i = din("sh05", [24, 1])
    W1sel_i = din("W1sel", [D, NSLOT, 4 * D], BF16)
    sW1_i = din("sW1T", [D, KS, 4 * D], BF16)
    b1selT_i = din("b1selT", [128, NSLOT * 4])
    sb1T_i = din("sb1T", [128, KS * 4])
    W2P_i = din("W2Psel", [D, NSLOT + KS, 4, 128], BF16)
    b2P_i = din("b2Psel", [D, NSLOT + KS])
    wgm_i = din("wgm", [12, NCH * 128], BF16)
    sel24_i = din("sel24", [128, 24 * 24], BF16)
    shifts_i = din("shifts", [24, 2 * 12], BF16)
    bb24_i = din("bb24", [24, D], BF16)
    gmask_i = din("gmask24", [24, GPC])
    hW1_i = din("hW1", [D, D], BF16)
    hb1_i = din("hb1_col", [D, 1])
    hW2_i = din("hW2col", [D, 1], BF16)
    hb2_i = din("hb2", [1, 1])

    out_o = nc.dram_tensor("out_row", [1, NC_NODES], F32, kind="ExternalOutput")

    with tile.TileContext(nc) as tc:
        with (
            tc.tile_pool(name="cp", bufs=1) as cp,
            tc.tile_pool(name="wk", bufs=3) as wk,
            tc.tile_pool(name="sm", bufs=4) as smp,
            tc.tile_pool(name="ps", bufs=1, space="PSUM") as ps,
        ):
            PS_BUFS = {"ph": 3, "pc": 3, "var": 1}
            _ld = [0]
            def load(ap_dram, shape, dt=F32):
                _ld[0] += 1
                t_ = cp.tile(shape, dt, tag=f"cst{_ld[0]}", name=f"cst{_ld[0]}")
                src_ap = ap_dram[:]
                if dt == F32R:
                    src_ap = src_ap.bitcast(F32R)
                nc.sync.dma_start(t_[:], src_ap)
                return t_

            # batch-0 slot data first in the DMA queue
            vembT = cp.tile([D, NC_NODES], BF16, tag="cvembT", name="vembT")
            nc.sync.dma_start(vembT[:, :NC_NODES // 2],
                              vembT_i[:, :NC_NODES // 2])
            W1 = cp.tile([D, NSLOT, 4 * D], BF16, tag="cW1", name="W1")
            nc.sync.dma_start(W1[:, :8, :], W1sel_i[:, :8, :])
            b1T = load(b1selT_i, [128, NSLOT * 4])
            sb1T = load(sb1T_i, [128, KS * 4])
            b2P = load(b2P_i, [D, NSLOT + KS])
            sW1 = load(sW1_i, [D, KS, 4 * D], BF16)
            W2P = cp.tile([D, NSLOT + KS, 4, 128], BF16, tag="cW2P",
                          name="W2P")
            nc.sync.dma_start(W2P[:, :8, :, :], W2P_i[:, :8, :, :])
            nc.sync.dma_start(W2P[:, NSLOT:, :, :], W2P_i[:, NSLOT:, :, :])
            wgm = load(wgm_i, [12, NCH, 128], BF16)
            shifts = load(shifts_i, [24, 2, 12], BF16)
            sel24 = load(sel24_i, [128, 24, 24], BF16)
            exlg = load(explog_i, [GPC, NE])
            msk = load(mask_i, [GPC, NE])
            Esel = load(Esel_i, [24, NE])
            Gsel = load(Gsel_i, [GPC, 24], F32R)
            sh05 = load(sh05_i, [24, 1])
            bb24 = load(bb24_i, [24, D], BF16)
            gmask = load(gmask_i, [24, GPC])
            hW1 = load(hW1_i, [D, D], BF16)
            hb1 = load(hb1_i, [D, 1])
            hW2 = load(hW2_i, [D, 1], BF16)
            hb2 = load(hb2_i, [1, 1])
            # batch-1 slot data at the tail of the DMA queue
            nc.sync.dma_start(vembT[:, NC_NODES // 2:],
                              vembT_i[:, NC_NODES // 2:])
            nc.sync.dma_start(W1[:, 8:, :], W1sel_i[:, 8:, :])
            nc.sync.dma_start(W2P[:, 8:NSLOT, :, :], W2P_i[:, 8:NSLOT, :, :])

            eps24 = cp.tile([24, 1], F32, name="eps24")
            nc.vector.memset(eps24[:], LN_EPS)

            acc = cp.tile([D, NC_NODES], F32, name="acc")
            cbS = cp.tile([128, NCH, 2, HF], BF16, name="cbS")
            out_sb = cp.tile([1, NC_NODES], F32, name="out_sb")

            # ---- route weights on device (exp(logits) comes from k1)
            sme = smp.tile([GPC, 1], F32, tag="sme", bufs=1, name="sme")
            nc.vector.tensor_reduce(sme[:], exlg[:], AX.X, ALU.add)
            rce = smp.tile([GPC, 1], F32, tag="rce", bufs=1, name="rce")
            nc.vector.reciprocal(rce[:], sme[:])
            w_sm = smp.tile([GPC, NE], F32, tag="w_sm", bufs=1, name="w_sm")
            nc.vector.tensor_scalar(w_sm[:], exlg[:], rce[:], None, ALU.mult)
            wm = smp.tile([GPC, NE], F32, tag="wm", bufs=1, name="wm")
            nc.vector.tensor_tensor(wm[:], w_sm[:], msk[:], ALU.mult)
            s2_ = smp.tile([GPC, 1], F32, tag="s2_", bufs=1, name="s2_")
            nc.vector.tensor_reduce(s2_[:], wm[:], AX.X, ALU.add)
            s2e = smp.tile([GPC, 1], F32, tag="s2e", bufs=1, name="s2e")
            nc.gpsimd.tensor_scalar(s2e[:], s2_[:], 1e-12, None, ALU.add)
            rc2 = smp.tile([GPC, 1], F32, tag="rc2", bufs=1, name="rc2")
            nc.vector.reciprocal(rc2[:], s2e[:])
            route = smp.tile([GPC, NE], F32, tag="route", bufs=1, name="route")
            nc.vector.tensor_scalar(route[:], wm[:], rc2[:], None, ALU.mult)
            route_r = smp.tile([GPC, NE], F32R, tag="route_r", bufs=1,
                               name="route_r")
            with nc.allow_low_precision(reason="route f32r view"):
                nc.vector.tensor_copy(route_r[:], route[:])

            pR2 = ps.tile([128, 512], F32, tag="pc", name="pR2",
                          bufs=PS_BUFS["pc"])
            nc.tensor.matmul(pR2[:24, :NE], Gsel[:], route_r[:],
                             start=True, stop=True)
            r2e = smp.tile([24, NE], F32, tag="r2e", bufs=1, name="r2e")
            nc.vector.tensor_tensor(r2e[:], pR2[:24, :NE], Esel[:], ALU.mult)
            wc24 = smp.tile([24, 1], F32, tag="wc24", bufs=1, name="wc24")
            nc.vector.tensor_reduce(wc24[:], r2e[:], AX.X, ALU.add)
            wcol24 = cp.tile([24, 1], F32, name="wcol24")
            nc.vector.tensor_tensor(wcol24[:], wc24[:], sh05[:], ALU.add)
            wcol24_bf = cp.tile([24, 1], BF16, name="wcol24_bf")
            nc.vector.tensor_copy(wcol24_bf[:], wcol24[:])
            wcolb = []
            for b in range(2):
                pwc = ps.tile([128, 512], F32, tag="pc", name="pwc",
                              bufs=PS_BUFS["pc"])
                nc.tensor.matmul(pwc[:12, :1], shifts[:, b, :], wcol24_bf[:],
                                 start=True, stop=True)
                wcb = cp.tile([12, 1], F32, name=f"wcb{b}")
                nc.vector.tensor_copy(wcb[:], pwc[:12, :1])
                wcolb.append(wcb)

            # per-graph LN bias columns: biasg = bb24^T @ (gmask * wcol24)
            wsel24 = smp.tile([24, GPC], BF16, tag="wsel", bufs=1,
                              name="wsel24")
            nc.vector.tensor_scalar(wsel24[:], gmask[:], wcol24[:], None,
                                    ALU.mult)
            pbg = ps.tile([128, 512], F32, tag="pc", name="pbg",
                          bufs=PS_BUFS["pc"])
            nc.tensor.matmul(pbg[:, :GPC], bb24[:], wsel24[:],
                             start=True, stop=True)
            biasg = cp.tile([D, GPC], F32, name="biasg")
            nc.vector.tensor_copy(biasg[:], pbg[:, :GPC])

            # ---- expert pipeline, two batches of 12 slots; pass B / head of
            # batch b overlaps pass A of batch b+1
            pvar = ps.tile([12, 2, 512], F32, tag="var", name="pvar",
                           bufs=PS_BUFS["var"])
            sq_t = [None] * NCH
            rstdw_t = [None, None]
            first = set()

            def emit_front(s, local, last_local):
                g, wi, b1i = SLOTS[s]
                off = g * PAD_G
                if b1i >= 0:
                    W1t = W1[:, b1i, :]
                    b1c = b1T[:, b1i * 4:(b1i + 1) * 4]
                else:
                    j = -1 - b1i
                    W1t = sW1[:, j, :]
                    b1c = sb1T[:, j * 4:(j + 1) * 4]
                hTns = []
                for h in range(2):
                    for c in range(4):
                        ph = ps.tile([128, HF], F32, tag="ph", name="ph",
                                     bufs=PS_BUFS["ph"])
                        nc.tensor.matmul(
                            ph[:], W1t[:, c * 128:(c + 1) * 128],
                            vembT[:, off + h * HF:off + (h + 1) * HF],
                            start=True, stop=True)
                        hTn = wk.tile([128, HF], BF16, tag="hTn", bufs=10,
                                      name="hTn")
                        nc.scalar.activation(hTn[:], ph[:], AF.Gelu,
                                             bias=b1c[:, c:c + 1])
                        hTns.append(hTn)
                if local >= 1:
                    emit_var(s - 1, local - 1, last_local)
                for h in range(2):
                    pc_ = ps.tile([128, HF], F32, tag="pc", name="pc_",
                                  bufs=PS_BUFS["pc"])
                    for c in range(4):
                        nc.tensor.matmul(pc_[:], W2P[:, wi, c, :],
                                         hTns[h * 4 + c][:],
                                         start=(c == 0), stop=(c == 3))
                    nc.vector.tensor_scalar(cbS[:, s, h, :], pc_[:],
                                            b2P[:, wi:wi + 1], None, ALU.add)
                sqt = wk.tile([128, 2, HF], BF16, tag="sq", bufs=3, name="sqt")
                nc.vector.tensor_tensor(sqt[:], cbS[:, s, :, :],
                                        cbS[:, s, :, :], ALU.mult)
                sq_t[s] = sqt

            def emit_var(s, local, last_local):
                for h in range(2):
                    nc.tensor.matmul(pvar[:, h, :HF], sel24[:, local, :12],
                                     sq_t[s][:, h, :],
                                     start=(local == 0),
                                     stop=(local == last_local))

            def emit_rstd(b):
                lnv = wk.tile([12, 2, HF], F32, tag="lnv", bufs=2, name="lnv")
                nc.scalar.activation(lnv[:], pvar[:, :, :HF],
                                     AF.Ln, bias=eps24[:12, :],
                                     scale=1.0 / D)
                rstd = wk.tile([12, 2, HF], BF16, tag="rstd", bufs=2,
                               name="rstd")
                nc.scalar.activation(rstd[:], lnv[:], AF.Exp, scale=-0.5)
                rstdw = wk.tile([12, 2, HF], BF16, tag="rstdw", bufs=2,
                                name="rstdw")
                nc.vector.tensor_scalar(rstdw[:], rstd[:],
                                        wcolb[b][:], None, ALU.mult)
                rstdw_t[b] = rstdw

            def passB_order(b):
                base = 12 * b
                order = []
                for k in range(TOPK):
                    for gl in range(2):
                        order.append(base + gl * TOPK + k)
                for j in range(KS):
                    for gl in range(2):
                        order.append(base + 8 + j * 2 + gl)
                return order

            def emit_passB(b, order):
                for s in order:
                    g, _, _ = SLOTS[s]
                    off = g * PAD_G
                    for h in range(2):
                        pA = ps.tile([128, HF], F32, tag="ph", name="pA",
                                     bufs=PS_BUFS["ph"])
                        nc.tensor.matmul(pA[:], wgm[:, s, :],
                                         rstdw_t[b][:, h, :],
                                         start=True, stop=True)
                        u = wk.tile([128, HF], F32, tag="u", bufs=4, name="u")
                        nc.vector.tensor_tensor(u[:], cbS[:, s, h, :], pA[:],
                                                ALU.mult)
                        asl = acc[:, off + h * HF:off + (h + 1) * HF]
                        if (off, h) not in first:
                            first.add((off, h))
                            nc.vector.tensor_tensor(
                                asl, u[:],
                                vembT[:, off + h * HF:off + (h + 1) * HF],
                                ALU.add)
                        else:
                            nc.vector.tensor_tensor(asl, asl, u[:], ALU.add)

            def emit_head(b):
                for g in (2 * b, 2 * b + 1):
                    emit_head_g(g)

            def emit_head_g(g):
                if True:
                    off = g * PAD_G
                    asl = acc[:, off:off + PAD_G]
                    nc.vector.tensor_scalar(asl, asl, biasg[:, g:g + 1], None,
                                            ALU.add)
                    acc_bf = wk.tile([128, PAD_G], BF16, tag="accbf", bufs=2,
                                     name="acc_bf")
                    nc.vector.tensor_copy(acc_bf[:], asl)
                    for h in range(2):
                        pr = ps.tile([128, HF], F32, tag="ph", name="pr",
                                     bufs=PS_BUFS["ph"])
                        nc.tensor.matmul(pr[:], hW1[:],
                                         acc_bf[:, h * HF:(h + 1) * HF],
                                         start=True, stop=True)
                        r_bf = wk.tile([128, HF], BF16, tag="rbf", bufs=3,
                                       name="r_bf")
                        nc.scalar.activation(r_bf[:], pr[:], AF.Relu,
                                             bias=hb1[:])
                        po = ps.tile([128, HF], F32, tag="pc", name="po",
                                     bufs=PS_BUFS["pc"])
                        nc.tensor.matmul(po[:1, :], hW2[:], r_bf[:],
                                         start=True, stop=True)
                        nc.vector.tensor_scalar(
                            out_sb[:, off + h * HF:off + (h + 1) * HF],
                            po[:1, :], hb2[:], None, ALU.add)
                    nc.sync.dma_start(out_o[:, off:off + PAD_G],
                                      out_sb[:, off:off + PAD_G])

            # batch 0 fronts
            for local in range(12):
                emit_front(local, local, 11)
            emit_var(11, 11, 11)
            emit_rstd(0)
            # batch 1 fronts, interleaved slot-by-slot with batch 0's pass B
            ord0 = passB_order(0)
            for local in range(12):
                emit_front(12 + local, local, 11)
                emit_passB(0, [ord0[local]])
            emit_var(23, 11, 11)
            emit_head(0)
            emit_rstd(1)
            ord1 = passB_order(1)
            for g in (2, 3):
                emit_passB(1, [s for s in ord1 if SLOTS[s][0] == g])
                emit_head_g(g)

    nc.compile()
    return nc


# ------------------------------------------------------------------- driver

_CACHE = {}
LAST_RES = [None, None]


def kernel(**inputs):
    return _run(inputs, trace=False)[0]


def timed_run(inputs):
    _, t1, t2 = _run(inputs, trace=True)
    return t1, t2


def _run(inputs, trace=False):
    inp = {k: np.asarray(v) for k, v in inputs.items()}
    f32 = lambda k: inp[k].astype(np.float32)
    i64 = lambda k: inp[k].astype(np.int64)

    assert np.all(inp["be"] == 0), "nonzero be not supported"

    edge_cons, edge_vars, batch_idx = i64("edge_cons"), i64("edge_vars"), i64("batch_idx")
    plan = _plan(edge_cons, edge_vars, f32("edge_attr"), batch_idx)
    CW = tuple(plan["CW"])

    key1 = ("k1", CW)
    if key1 not in _CACHE:
        _CACHE[key1] = _build_k1(list(CW))
    nc1 = _CACHE[key1]

    P_bf = (np.eye(128) - 1.0 / 128).astype(np.float32).astype(BF)
    sel24 = _sel24()
    onesm = _onesm()

    c_feat = f32("c_feat")
    v_feat = f32("v_feat")
    counts = plan["counts"]
    ntot = plan["ntot"]

    dW2, sW2 = f32("dW2"), f32("sW2")
    W2all = np.ascontiguousarray(
        np.concatenate([dW2, sW2], axis=0).reshape(
            NE + KS, 4, 128, 128).transpose(3, 0, 1, 2)).astype(BF)
    b2allT = np.ascontiguousarray(
        np.concatenate([f32("db2"), f32("sb2")], axis=0).T).astype(BF)

    in1 = []
    for c in range(NCORE):
        nos = plan["node_of_slot"][c]
        vfT = np.zeros((VF, NC_NODES), np.float32)
        real = nos >= 0
        vfT[:, real] = v_feat[nos[real]].T
        cnt = counts[c].astype(np.float32)
        padc = (PAD_G - counts[c]).astype(np.float32)
        ecidx = plan["ecidx"][c]
        used = plan["used"][c]
        cfa = np.zeros((128 * ntot, CF1), np.float32)
        cfa[used, :CF] = c_feat[ecidx[used]]
        cfa[used, CF] = 1.0
        m = dict(
            ecf=np.ascontiguousarray(
                cfa.reshape(ntot, 128, CF1).transpose(1, 0, 2).reshape(
                    128, ntot * CF1)).astype(BF),
            oea=_build_oea(plan, c),
            Wca=np.concatenate([f32("Wc"), f32("bc").reshape(1, D)],
                               axis=0).astype(BF),
            Wv=f32("Wv"), bv_col=f32("bv").reshape(D, 1),
            vfeatT=vfT,
            We_col=f32("We").reshape(D, 1),
            lng_col=f32("ln_g").reshape(D, 1), lnb_col=f32("ln_b").reshape(D, 1),
            P_bf=P_bf,
            WqT=np.ascontiguousarray(f32("Wq").T),
            tokKT=np.ascontiguousarray(f32("tokK").T),
            bq_col=f32("bq").reshape(TD, 1),
            tokV=f32("tokV").astype(BF),
            Wg_r=np.ascontiguousarray(f32("Wg").reshape(2, D, NE).transpose(1, 0, 2)),
            bg_col=f32("bg").reshape(NE, 1), eb_col=f32("ebias").reshape(NE, 1),
            alpha11=f32("alpha").reshape(1, 1).astype(BF),
            sel24=sel24, onesm=onesm,
            padc4=np.tile(padc[None, :], (128, 1)),
            invc4=np.tile((1.0 / np.maximum(cnt, 1.0))[None, :], (128, 1)),
            negpadc=(-padc).reshape(1, GPC).astype(BF),
            W2all=W2all, b2allT=b2allT,
        )
        in1.append(m)

    res1 = run_bass_kernel_spmd(nc1, in1, CORE_IDS, trace=trace)
    LAST_RES[0] = res1

    explog = np.concatenate(
        [np.asarray(res1.results[c]["explogT"]).T.astype(np.float32)
         for c in range(NCORE)], axis=0)                          # [B, NE]
    top_idx = np.argsort(-explog, axis=1, kind="stable")[:, :TOPK]  # [B, 4]
    mask = np.zeros((B, NE), np.float32)
    np.put_along_axis(mask, top_idx, 1.0, axis=1)

    if "k2" not in _CACHE:
        _CACHE["k2"] = _build_k2()
    nc2 = _CACHE["k2"]

    dW1 = f32("dW1")
    dg, dbb = f32("dg"), f32("dbb")
    sW1 = f32("sW1")
    sg, sbb = f32("sg"), f32("sbb")

    shifts_c = np.zeros((24, 2, 12), np.float32)
    for b in range(2):
        for i in range(12):
            shifts_c[12 * b + i, b, i] = 1.0
    shifts_c = shifts_c.reshape(24, 2 * 12).astype(BF)
    in2 = []
    for c in range(NCORE):
        # dedicated experts in packed (batch-major) slot order
        sel = np.array([top_idx[c * GPC + g, k] for g, k in DED_GK])  # [16]
        Esel24 = np.zeros((24, NE), np.float32)
        Gsel24 = np.zeros((GPC, 24), np.float32)
        sh05 = np.zeros((24, 1), np.float32)
        gmask24 = np.zeros((24, GPC), np.float32)
        bb24 = np.zeros((24, D), np.float32)
        wgm = np.zeros((12, NCH, 128), np.float32)
        nded = 0
        for s, (g, wi, b1i) in enumerate(SLOTS):
            gmask24[s, g] = 1.0
            if b1i >= 0:
                e = sel[nded]; nded += 1
                Esel24[s, e] = 1.0
                Gsel24[g, s] = 1.0
                bb24[s] = dbb[e]
                wgm[s % 12, s, :] = dg[e]
            else:
                j = -1 - b1i
                sh05[s, 0] = 1.0 / KS
                bb24[s] = sbb[j]
                wgm[s % 12, s, :] = sg[j]
        W1s = dW1[sel]                                  # [16, 128, 512]
        b1s = f32("db1")[sel]                           # [16, 512]
        W2Pall = np.asarray(res1.results[c]["W2Pall"]).reshape(D, NE + KS,
                                                               4, 128)
        b2Pall = np.asarray(res1.results[c]["b2Pall"]).astype(np.float32)
        slotmap = np.concatenate([sel, NE + np.arange(KS)])
        W2Psel = np.ascontiguousarray(W2Pall[:, slotmap])
        b2Psel = np.ascontiguousarray(b2Pall[:, slotmap])
        m = dict(
            vembT_bf=np.asarray(res1.results[c]["vembT"]).astype(BF),
            explog_nm=explog[c * GPC:(c + 1) * GPC],
            mask_nm=mask[c * GPC:(c + 1) * GPC],
            Esel24=Esel24, Gsel24=Gsel24, sh05=sh05,
            W1sel=np.ascontiguousarray(W1s.transpose(1, 0, 2)).astype(BF),
            sW1T=np.ascontiguousarray(sW1.transpose(1, 0, 2)).astype(BF),
            b1selT=np.ascontiguousarray(
                b1s.reshape(NSLOT, 4, 128).transpose(2, 0, 1).reshape(
                    128, NSLOT * 4)),
            sb1T=np.ascontiguousarray(
                f32("sb1").reshape(KS, 4, 128).transpose(2, 0, 1).reshape(
                    128, KS * 4)),
            W2Psel=W2Psel, b2Psel=b2Psel,
            wgm=wgm.reshape(12, NCH * 128).astype(BF),
            sel24=sel24, shifts=shifts_c,
            bb24=bb24.astype(BF),
            gmask24=gmask24,
            hW1=f32("hW1").astype(BF), hb1_col=f32("hb1").reshape(D, 1),
            hW2col=f32("hW2").reshape(D, 1).astype(BF),
            hb2=f32("hb2").reshape(1, 1),
        )
        in2.append(m)

    res2 = run_bass_kernel_spmd(nc2, in2, CORE_IDS, trace=trace)
    LAST_RES[1] = res2

    out = np.zeros(N, np.float32)
    for c in range(NCORE):
        row = np.asarray(res2.results[c]["out_row"],
                         dtype=np.float32).reshape(-1)
        nos = plan["node_of_slot"][c]
        real = nos >= 0
        out[nos[real]] = row[real]
    return out, res1.exec_time_ns, res2.exec_time_ns
